# revision 58
# baseline (speedup 1.0000x reference)
"""Trainium2 8-core kernel for a dense pre-norm transformer block.

Reference: h=LN1(x); qkv=h@w_qkv; causal MHA (16 heads, Dh=64);
x+=o@w_out; h2=LN2(x); x+=gelu(h2@w1+b1)@w2+b2.

Sharding (Megatron TP-8 + sequence-parallel residual):
  - heads 2c,2c+1 on core c (w_qkv column-shard, w_out row-shard)
  - MLP hidden 512 per core (w1 column-shard, w2 row-shard)
  - residual stream token-sharded: core c owns the four strided pieces
    {q*1024 + c*128 .. +128}, q=0..3. RS1 runs as four quarter-sized
    ReduceScatters fired as soon as the out-projection of each quarter
    is done (the first two hide under attention of batch 1); the
    AllGather runs per half; RS2 runs as four quarters, the first three
    hidden under remaining MLP work.
  - LN1 stats (mean/rstd) are computed shard-locally with bn_stats and
    exchanged via a tiny AllGather at kernel start; LN gains/biases are
    folded into w_qkv/w1 host-side; LN1 mean-subtraction is folded into
    the qkv matmul as a rank-1 (-colsum(w) x mean) accumulation and the
    rstd scaling is applied to the matmul output.

Compute dtype: bf16 operands, fp32 PSUM accumulation, fp32 residual.
Attention scores are computed transposed ST=[k_pos, q_pos]; both heads
run concurrently in the PE array via tile_position row-packing; softmax
denominator comes from a ones-column appended to V; causality via 4
static [128,512] masks.
"""
import numpy as np

import concourse.bass as bass
import concourse.mybir as mybir
import concourse.tile as tile
from concourse import bacc
from concourse import bass_utils
from concourse.masks import make_identity

F32 = mybir.dt.float32
BF = mybir.dt.bfloat16
F8 = mybir.dt.float8e4
AF = mybir.ActivationFunctionType
DR = mybir.MatmulPerfMode.DoubleRow
WSC = 32.0  # fp8 weight prescale (avoids e4m3 subnormals for 0.02-scale w)

NCORES = 8
B, L, D = 2, 2048, 1024
T = B * L              # 4096 tokens
TSH = T // NCORES      # 512 tokens per core (4 pieces of 128)
DH = 64                # head dim
HL = 2                 # heads per core
DLOC = HL * DH         # 128 local head features
MLPH = 4096 // NCORES  # 512 local hidden
LN_EPS = 1e-5
NT = T // 512          # 8 token tiles of 512
ND = D // 128          # 8 feature chunks
QT = L // 512          # 4 q-tiles per batch

_CACHE = {}


def build():
    if "nc" in _CACHE:
        return _CACHE["nc"]
    nc = bacc.Bacc("TRN2", target_bir_lowering=False, debug=False,
                   num_devices=NCORES)

    xt_in = nc.dram_tensor("xt", [D, T], F8, kind="ExternalInput")
    xsh_in = nc.dram_tensor("xsh", [TSH, D], F32, kind="ExternalInput")
    wqkv_in = nc.dram_tensor("wqkv", [128, ND // 2, 3, 2, 128], F8,
                             kind="ExternalInput")
    nws_in = nc.dram_tensor("nws", [3 * DLOC, 1], F32, kind="ExternalInput")
    bqkv_in = nc.dram_tensor("bqkv", [3 * DLOC, 1], F32, kind="ExternalInput")
    wout_in = nc.dram_tensor("wout", [DLOC, D], BF, kind="ExternalInput")
    w1_in = nc.dram_tensor("w1", [D, 4 * D], BF, kind="ExternalInput")
    b1g_in = nc.dram_tensor("b1g", [4 * D, 1], F32, kind="ExternalInput")
    w2_in = nc.dram_tensor("w2", [4 * D, D], BF, kind="ExternalInput")
    b2b_in = nc.dram_tensor("b2b", [128, D], BF, kind="ExternalInput")
    masks_in = nc.dram_tensor("masks", [4, 128, 512], F8, kind="ExternalInput")
    out_ext = nc.dram_tensor("out", [TSH, D], F32, kind="ExternalOutput")

    rg = [list(range(NCORES))]

    with tile.TileContext(nc) as tc:
        with (
            tc.tile_pool(name="const", bufs=1) as const,
            tc.tile_pool(name="wpool", bufs=1) as wpool,
            tc.tile_pool(name="dram", bufs=1, space="DRAM") as dram,
        ):
            # ---- DRAM scratch for collectives ----
            st_ag_in = dram.tile([8, 128], BF)       # (piece q, mean/rstd)
            st_ag_out = dram.tile([64, 128], BF, addr_space="Shared")
            rs1_in = [dram.tile([1024, D], BF, name=f"rs1_in{q}")
                      for q in range(4)]
            rs1_out = [dram.tile([128, D], BF, name=f"rs1_out{q}")
                       for q in range(4)]

            warm_in = dram.tile([8, 16], BF)
            warm_out = dram.tile([64, 16], BF, addr_space="Shared")

            # ---- constants ----
            ident = const.tile([128, 128], F32)
            make_identity(nc, ident[:])
            ident_bf = const.tile([128, 128], BF)
            make_identity(nc, ident_bf[:])
            ones_row = const.tile([1, 128], BF)
            nc.vector.memset(ones_row[:], 1.0)
            sc_row = const.tile([1, 128], BF)
            nc.vector.memset(sc_row[:], 1.0 / WSC)
            eps128 = const.tile([128, 1], F32)
            nc.vector.memset(eps128[:], LN_EPS)
            masks_sb = [const.tile([128, 512], F8, name=f"mask{m}")
                        for m in range(4)]

            def load_masks():
                for m in range(4):
                    nc.sync.dma_start(masks_sb[m][:], masks_in.ap()[m])

            # ---- weights resident in SBUF ----
            # wqkv_sb[r, p, m, i, c] = WSC * w_eff[p*256 + i*128 + r,
            #                                      m*128 + c], fp8
            # (pair k-tiles contiguous for dual-fp8 ldweights)
            wqkv_sb = wpool.tile([128, ND // 2, 3, 2, 128], F8, name="wqkv8")
            nc.sync.dma_start(wqkv_sb[:], wqkv_in.ap())
            nws_sb = []
            for m in range(3):
                nt_ = wpool.tile([128, 1], F32, name=f"nws{m}")
                nc.sync.dma_start(nt_[:],
                                  nws_in.ap()[m * 128:(m + 1) * 128, :])
                nws_sb.append(nt_)
            bqkv_sb = []
            for m in range(3):
                bt = wpool.tile([128, 1], F32, name=f"bqkv{m}")
                nc.sync.dma_start(bt[:], bqkv_in.ap()[m * 128:(m + 1) * 128, :])
                bqkv_sb.append(bt)
            NM = 4 * D // 128       # 32 hidden chunks (full MLP per core)
            wout_sb = wpool.tile([DLOC, D], BF)
            b2b_sb = wpool.tile([128, D], BF, name="b2b")

            def load_late_weights():
                nc.sync.dma_start(wout_sb[:], wout_in.ap())
                nc.sync.dma_start(b2b_sb[:], b2b_in.ap())

            resid_pool_cm = tc.tile_pool(name="resid", bufs=1)
            resid_pool = resid_pool_cm.__enter__()
            xsv = resid_pool.tile([128, 4, D], F32)   # my shard of x
            x2_sb = resid_pool.tile([128, 4, D], F32)
            h2T = resid_pool.tile([128, ND, 512], BF)     # LN2(x2)^T shard
            g1_sb = resid_pool.tile([128, NM, 512], BF)   # gelu acts [h, tok]

            # ========== stage 0: shard-local LN1 stats + tiny AG ==========
            s0_cm = tc.tile_pool(name="s0", bufs=2)
            s0 = s0_cm.__enter__()
            ps0_cm = tc.tile_pool(name="ps0", bufs=2, space="PSUM")
            ps0 = ps0_cm.__enter__()
            # absorb first-collective init latency with a no-dep dummy
            wtile = s0.tile([8, 16], BF, tag="wtile", name="wtile")
            nc.vector.memset(wtile[:], 0.0)
            nc.scalar.dma_start(warm_in[:], wtile[:])
            nc.gpsimd.collective_compute(
                "AllGather", mybir.AluOpType.bypass, replica_groups=rg,
                ins=[warm_in[:].opt()], outs=[warm_out[:].opt()])
            for q in range(4):
                nc.gpsimd.dma_start(
                    xsv[:, q, :], xsh_in.ap()[q * 128:(q + 1) * 128, :])
                stats = s0.tile([128, 2, 6], F32, tag="stats", name="stats")
                xv = xsv[:, q, :].rearrange("p (s f) -> p s f", s=2)
                for s in range(2):
                    nc.vector.bn_stats(stats[:, s, :], xv[:, s, :])
                mv = s0.tile([128, 2], F32, tag="mv", name="mv")
                nc.vector.bn_aggr(mv[:], stats[:])
                rstd0 = s0.tile([128, 1], F32, tag="rstd0", name="rstd0")
                nc.scalar.activation(rstd0[:], mv[:, 1:2], AF.Sqrt,
                                     bias=eps128[:])
                nc.vector.reciprocal_approx_fast(rstd0[:], rstd0[:])
                st2 = s0.tile([128, 2], BF, tag="st2", name="st2")
                nc.vector.tensor_copy(st2[:, 0:1], mv[:, 0:1])
                nc.vector.tensor_copy(st2[:, 1:2], rstd0[:])
                stp = ps0.tile([2, 128], BF, tag="stp", name="stp")
                nc.tensor.transpose(stp[:], st2[:], ident_bf[:])
                sts = s0.tile([2, 128], BF, tag="sts", name="sts")
                nc.vector.tensor_copy(sts[:], stp[:])
                nc.scalar.dma_start(st_ag_in[2 * q:2 * q + 2, :], sts[:])
            nc.gpsimd.collective_compute(
                "AllGather", mybir.AluOpType.bypass, replica_groups=rg,
                ins=[st_ag_in[:].opt()], outs=[st_ag_out[:].opt()])
            ps0_cm.__exit__(None, None, None)
            s0_cm.__exit__(None, None, None)

            # st_ag_out rows: c*8 + q*2 + {0:mean, 1:rstd}
            st_view = st_ag_out[:].rearrange("(c x) f -> c x f", x=8)

            praws = {}

            def do_s1_mm(tt):
                q4, h4 = tt // 2, tt % 2
                xts = s1_x.tile([128, ND, 512], F8, tag="xts")
                for d in range(ND):
                    eng = (nc.sync, nc.scalar, nc.gpsimd)[d % 3]
                    eng.dma_start(
                        xts[:, d, :],
                        xt_in.ap()[d * 128:(d + 1) * 128,
                                   tt * 512:(tt + 1) * 512])
                praws[tt] = (None, None, [])
                for m in range(3):
                    ps_q = ps_qkv.tile([128, 512], F32, tag="ps_q",
                                       name="ps_q")
                    for p in range(ND // 2):
                        nc.tensor.matmul(
                            ps_q[:],
                            wqkv_sb[:, p, m, :, :],
                            xts[:, 2 * p:2 * p + 2, :],
                            start=(p == 0), stop=(p == ND // 2 - 1),
                            perf_mode=DR)
                    praw = praw_pool.tile([128, 512], BF, tag="praw",
                                          name="praw")
                    nc.scalar.copy(praw[:], ps_q[:])
                    praws[tt][2].append(praw)

            def do_s1_fin(tt):
                q4, h4 = tt // 2, tt % 2
                _, _, praw3 = praws[tt]
                mean_bf = s1_stat.tile([1, 4, 128], BF, tag=f"mean_bf{tt}",
                                       name=f"mean_bf{tt}")
                rstd_bf = s1_stat.tile([1, 4, 128], BF, tag=f"rstd_bf{tt}",
                                       name=f"rstd_bf{tt}")
                nc.gpsimd.dma_start(
                    mean_bf[:], st_view[4 * h4:4 * h4 + 4, 2 * q4, :])
                nc.gpsimd.dma_start(
                    rstd_bf[:], st_view[4 * h4:4 * h4 + 4, 2 * q4 + 1, :])
                mean_v = mean_bf[:].rearrange("p a f -> p (a f)")
                rstd_v = rstd_bf[:].rearrange("p a f -> p (a f)")
                mr = s1_stat.tile([1, 512], BF, tag="mr", name="mr")
                nc.vector.tensor_mul(mr[:], mean_v, rstd_v)
                rstd_b = ps_st.tile([128, 512], F32, tag="pst",
                                    name="rstd_b")
                nc.tensor.matmul(rstd_b[:], sc_row[:], rstd_v,
                                 start=True, stop=True)
                rstd_bc = s1_tmp.tile([128, 512], BF, tag="rstd_bc")
                if tt >= 4:
                    nc.scalar.copy(rstd_bc[:], rstd_b[:])
                else:
                    nc.vector.tensor_copy(rstd_bc[:], rstd_b[:])
                mr_b = ps_st.tile([128, 512], F32, tag="pst", name="mr_b")
                nc.tensor.matmul(mr_b[:], ones_row[:], mr[:],
                                 start=True, stop=True)
                for m in range(3):
                    u = s1_tmp.tile([128, 512], BF, tag="pre", name="u")
                    nc.vector.tensor_mul(u[:], praw3[m][:], rstd_bc[:])
                    pre = s1_tmp.tile([128, 512], BF, tag="pre2",
                                      name="pre2")
                    nc.vector.scalar_tensor_tensor(
                        out=pre[:], in0=mr_b[:], scalar=nws_sb[m][:],
                        in1=u[:], op0=mybir.AluOpType.mult,
                        op1=mybir.AluOpType.add)
                    nc.vector.tensor_scalar(
                        out=qkvT[m][:, tt * 512:(tt + 1) * 512], in0=pre[:],
                        scalar1=bqkv_sb[m][:], scalar2=None,
                        op0=mybir.AluOpType.add)
                del praws[tt]
            # ============ stage 2/3/4 pools ============
            s2_vaug_cm = tc.tile_pool(name="s2_vaug", bufs=1)
            s2_vaug = s2_vaug_cm.__enter__()
            s2_exp_cm = tc.tile_pool(name="s2_exp", bufs=2)
            s2_exp = s2_exp_cm.__enter__()
            s2_misc_cm = tc.tile_pool(name="s2_misc", bufs=1)
            s2_misc = s2_misc_cm.__enter__()
            s3_r1_cm = tc.tile_pool(name="s3_r1", bufs=2)
            s3_r1 = s3_r1_cm.__enter__()
            s4_t_cm = tc.tile_pool(name="s4_t", bufs=1)
            s4_t = s4_t_cm.__enter__()
            ps_st_cm = tc.tile_pool(name="ps_st", bufs=3, space="PSUM")
            ps_st = ps_st_cm.__enter__()
            ps_o_cm = tc.tile_pool(name="ps_o", bufs=1, space="PSUM")
            ps_o = ps_o_cm.__enter__()
            ps_vt_cm = tc.tile_pool(name="ps_vt", bufs=1, space="PSUM")
            ps_vt = ps_vt_cm.__enter__()

            # persistent activations
            attn_pool_cm = tc.tile_pool(name="attn", bufs=1)
            attn_pool = attn_pool_cm.__enter__()
            qkvT = []
            for m in range(3):
                t_ = attn_pool.tile([128, T], BF, name=f"qkvT{m}")
                qkvT.append(t_)
            oT = attn_pool.tile([128, T], BF)

            # ================= stage 1 pools (popped mid-kernel) ==========
            s1_x_cm = tc.tile_pool(name="s1_x", bufs=2)
            s1_x = s1_x_cm.__enter__()
            s1_tmp_cm = tc.tile_pool(name="s1_tmp", bufs=3)
            s1_tmp = s1_tmp_cm.__enter__()
            s1_stat_cm = tc.tile_pool(name="s1_stat", bufs=1)
            s1_stat = s1_stat_cm.__enter__()
            ps_qkv_cm = tc.tile_pool(name="ps_qkv", bufs=2, space="PSUM")
            ps_qkv = ps_qkv_cm.__enter__()
            praw_pool_cm = tc.tile_pool(name="s1_praw", bufs=12)
            praw_pool = praw_pool_cm.__enter__()

            vaugs = {}

            def do_vaug(b, tl):
                """V-transposes for 512-token tile tl (4 k-chunks) of batch b."""
                tok0 = b * L
                if b not in vaugs:
                    vaug = s2_vaug.tile([128, HL, L // 256, 2, DH + 16], F8,
                                        tag=f"vaug{b}", name=f"vaug{b}")
                    nc.vector.memset(vaug[:, :, :, :, DH:DH + 1], 1.0)
                    nc.vector.memset(vaug[:, :, :, :, DH + 1:DH + 16], 0.0)
                    vaugs[b] = vaug
                vaug = vaugs[b]
                for hl in range(HL):
                    hrow = hl * DH
                    vT_u = qkvT[2][hrow:hrow + DH, tok0:tok0 + L]
                    for kc in range(4 * tl, 4 * tl + 4):
                        pv = ps_vt.tile([128, DH], BF, tag="pv",
                                        name="pv")
                        nc.tensor.transpose(
                            pv[:], vT_u[:, kc * 128:(kc + 1) * 128],
                            ident_bf[hrow:hrow + DH, hrow:hrow + DH])
                        if b == 0:
                            nc.scalar.copy(
                                vaug[:, hl, kc // 2, kc % 2, 0:DH], pv[:])
                        else:
                            nc.vector.tensor_copy(
                                vaug[:, hl, kc // 2, kc % 2, 0:DH], pv[:])

            def do_attn(b, js):
                tok0 = b * L
                vaug = vaugs[b]
                for j in js:
                    nk = 4 * (j + 1)
                    po = [ps_o.tile([DH + 16, 512], F32, tag=f"po{hl}",
                                    name=f"po{hl}") for hl in range(HL)]
                    for kcp in range(nk // 2):
                        kc0 = 2 * kcp
                        dm0 = kc0 - (nk - 4)
                        col0 = 128 * dm0 if dm0 > 0 else 0
                        w = 512 - col0
                        ests = []
                        for hl in range(HL):
                            hrow = hl * DH
                            qsl = qkvT[0][hrow:hrow + DH,
                                          tok0 + j * 512 + col0:
                                          tok0 + (j + 1) * 512]
                            est = s2_exp.tile([128, 2, 512], F8,
                                              tag=f"est{hl}", name=f"est{hl}")
                            for i in range(2):
                                kc = kc0 + i
                                dm = kc - (nk - 4)
                                ksl = qkvT[1][hrow:hrow + DH,
                                              tok0 + kc * 128:
                                              tok0 + (kc + 1) * 128]
                                pst = ps_st.tile([128, 512], F32, tag="pst",
                                                 name="pst")
                                nc.tensor.matmul(pst[:, :w], ksl, qsl,
                                                 start=True, stop=True,
                                                 tile_position=(hrow, 0))
                                if dm >= 0:
                                    mw = 128 * (dm + 1) - col0
                                    nc.vector.tensor_add(
                                        pst[:, :mw], pst[:, :mw],
                                        masks_sb[dm][:, col0:col0 + mw])
                                nc.scalar.activation(est[:, i, :w],
                                                     pst[:, :w],
                                                     AF.Exp, scale=0.125)
                            ests.append(est)
                        for hl in range(HL):
                            nc.tensor.matmul(po[hl][:, col0:],
                                             vaug[:, hl, kcp, :, :],
                                             ests[hl][:, :, :w],
                                             start=(kcp == 0),
                                             stop=(kcp == nk // 2 - 1),
                                             perf_mode=DR)
                    for hl in range(HL):
                        hrow = hl * DH
                        den = s2_misc.tile([1, 512], F32, tag="den",
                                           name="den")
                        nc.vector.tensor_copy(den[:], po[hl][DH:DH + 1, :])
                        rec1 = s2_misc.tile([1, 512], F32, tag="rec1",
                                            name="rec1")
                        nc.vector.reciprocal_approx_fast(rec1[:], den[:])
                        rec1b = s2_misc.tile([1, 512], BF, tag="rec1b",
                                             name="rec1b")
                        nc.vector.tensor_copy(rec1b[:], rec1[:])
                        rec_b = ps_vt.tile([64, 512], F32, tag="pv",
                                           name="rec_b")
                        nc.tensor.matmul(rec_b[:], ones_row[0:1, 0:64],
                                         rec1b[:], start=True, stop=True)
                        rec_sb = s2_misc.tile([64, 512], BF, tag="rec_sb",
                                              name="rec_sb")
                        nc.vector.tensor_copy(rec_sb[:], rec_b[:])
                        nc.vector.tensor_mul(
                            oT[hrow:hrow + DH,
                               tok0 + j * 512:tok0 + (j + 1) * 512],
                            po[hl][0:DH, :], rec_sb[:])

            def do_oproj(q):
                """out-projection for quarter q (tokens q*1024..+1024) + RS."""
                for tch in range(8):
                    row0 = q * 1024 + tch * 128
                    r1 = s3_r1.tile([128, D], BF, tag="r1", name="r1")
                    for n in range(2):
                        pop = ps_st.tile([128, 512], F32, tag="pst",
                                         name="pop")
                        nc.tensor.matmul(pop[:], oT[:, row0:row0 + 128],
                                         wout_sb[:, n * 512:(n + 1) * 512],
                                         start=True, stop=True)
                        if n == 0:
                            nc.vector.tensor_copy(
                                r1[:, n * 512:(n + 1) * 512], pop[:])
                        else:
                            nc.scalar.copy(
                                r1[:, n * 512:(n + 1) * 512], pop[:])
                    nc.gpsimd.dma_start(
                        rs1_in[q][tch * 128:(tch + 1) * 128, :], r1[:])
                nc.gpsimd.collective_compute(
                    "ReduceScatter", mybir.AluOpType.add, replica_groups=rg,
                    ins=[rs1_in[q][:].opt()], outs=[rs1_out[q][:].opt()])

            def do_s4(q):
                """residual + LN2 + transpose for my piece of quarter q."""
                r1s = s4_t.tile([128, D], BF, tag="r1s", name="r1s")
                nc.sync.dma_start(r1s[:], rs1_out[q][:])
                nc.vector.tensor_add(x2_sb[:, q, :], xsv[:, q, :], r1s[:])
                stats = s4_t.tile([128, 2, 6], F32, tag="stats", name="stats")
                x2v = x2_sb[:, q, :].rearrange("p (s f) -> p s f", s=2)
                for s in range(2):
                    nc.vector.bn_stats(stats[:, s, :], x2v[:, s, :])
                mv = s4_t.tile([128, 2], F32, tag="mv", name="mv")
                nc.vector.bn_aggr(mv[:], stats[:])
                rstd2 = s4_t.tile([128, 1], F32, tag="rstd2", name="rstd2")
                nc.scalar.activation(rstd2[:], mv[:, 1:2], AF.Sqrt,
                                     bias=eps128[:])
                nc.vector.reciprocal_approx_fast(rstd2[:], rstd2[:])
                h2 = s4_t.tile([128, D], F32, tag="h2", name="h2")
                nc.vector.tensor_scalar(
                    out=h2[:], in0=x2_sb[:, q, :], scalar1=mv[:, 0:1],
                    scalar2=rstd2[:], op0=mybir.AluOpType.subtract,
                    op1=mybir.AluOpType.mult)
                for d in range(ND):
                    pt = ps_vt.tile([128, 128], F32, tag="pv", name="pt")
                    nc.tensor.transpose(
                        pt[:], h2[:, d * 128:(d + 1) * 128], ident[:])
                    nc.vector.tensor_copy(h2T[:, d, q * 128:(q + 1) * 128],
                                          pt[:])

            # ---------------- pipelined schedule (front) ----------------
            for tt in range(4):
                do_s1_mm(tt)
            load_masks()
            load_late_weights()
            do_s1_fin(0)
            do_s1_mm(4)
            do_vaug(0, 0)
            do_s1_fin(1)
            do_s1_mm(5)
            do_vaug(0, 1)
            do_s1_fin(2)
            do_s1_mm(6)
            do_vaug(0, 2)
            do_s1_fin(3)
            do_s1_mm(7)
            do_vaug(0, 3)
            do_attn(0, (0, 1))
            for tt in range(4, NT):
                do_s1_fin(tt)
                do_vaug(1, tt - 4)
            do_oproj(0)
            do_attn(0, (2, 3))

            # s1 done: free its SBUF/PSUM, bring in the full w1 for DP-MLP
            praw_pool_cm.__exit__(None, None, None)
            for cm in (ps_qkv_cm, s1_stat_cm, s1_tmp_cm, s1_x_cm):
                cm.__exit__(None, None, None)
            w1p_cm = tc.tile_pool(name="w1p", bufs=1)
            w1p = w1p_cm.__enter__()
            w1_sb = [w1p.tile([128, 4 * D], BF, name=f"w1_{d}")
                     for d in range(ND)]
            b1g_sb = w1p.tile([128, NM], F32, name="b1g")
            nc.sync.dma_start(
                b1g_sb[:],
                b1g_in.ap().rearrange("(m r) o -> r (m o)", r=128))
            for d in range(ND):
                nc.gpsimd.dma_start(w1_sb[d][:],
                                    w1_in.ap()[d * 128:(d + 1) * 128, :])
            ps_m1_cm = tc.tile_pool(name="ps_m1", bufs=2, space="PSUM")
            ps_m1 = ps_m1_cm.__enter__()

            def do_mlp1(qp, ms):
                """MLP1+GELU for token half qp (256 cols), hidden chunks ms."""
                c0 = qp * 256
                for m in ms:
                    pm1 = ps_m1.tile([128, 256], F32, tag="pm1", name="pm1")
                    for d in range(ND):
                        nc.tensor.matmul(
                            pm1[:], w1_sb[d][:, m * 128:(m + 1) * 128],
                            h2T[:, d, c0:c0 + 256], start=(d == 0),
                            stop=(d == ND - 1))
                    nc.scalar.activation(g1_sb[:, m, c0:c0 + 256], pm1[:],
                                         AF.Gelu, bias=b1g_sb[:, m:m + 1])

            do_oproj(1)
            do_attn(1, (2, 3))   # hides RS1_0 + RS1_1
            do_oproj(3)
            do_s4(0)
            do_s4(1)
            do_attn(1, (0, 1))   # hides RS1_3
            do_oproj(2)
            do_mlp1(0, range(NM))      # hides RS1_2
            do_s4(3)
            do_s4(2)
            do_mlp1(1, range(NM))

            for cm in (ps_m1_cm, w1p_cm, attn_pool_cm,
                       ps_vt_cm, ps_o_cm, ps_st_cm, s4_t_cm,
                       s3_r1_cm, s2_misc_cm, s2_exp_cm, s2_vaug_cm):
                cm.__exit__(None, None, None)

            # ---- stage 6: MLP2 (m-major, all 8 PSUM banks accumulate) ----
            s6_w2_cm = tc.tile_pool(name="s6_w2", bufs=8)
            s6_w2 = s6_w2_cm.__enter__()
            s6_o_cm = tc.tile_pool(name="s6_o", bufs=2)
            s6_o = s6_o_cm.__enter__()
            ps_m2_cm = tc.tile_pool(name="ps_m2", bufs=1, space="PSUM")
            ps_m2 = ps_m2_cm.__enter__()

            pm2 = [ps_m2.tile([128, 1024], F32, tag=f"pm2_{tc_}",
                              name=f"pm2_{tc_}") for tc_ in range(4)]
            for m in range(NM):
                w2c = s6_w2.tile([128, D], BF, tag="w2c", name="w2c")
                nc.sync.dma_start(w2c[:],
                                   w2_in.ap()[m * 128:(m + 1) * 128, :])
                for tc_ in range(4):
                    for n2 in range(2):
                        nc.tensor.matmul(
                            pm2[tc_][:, n2 * 512:(n2 + 1) * 512],
                            g1_sb[:, m, tc_ * 128:(tc_ + 1) * 128],
                            w2c[:, n2 * 512:(n2 + 1) * 512],
                            start=(m == 0), stop=(m == NM - 1))
            for tc_ in range(4):
                ot = s6_o.tile([128, D], F32, tag="ot", name="ot")
                nc.vector.tensor_add(ot[:], x2_sb[:, tc_, :], pm2[tc_][:])
                nc.vector.tensor_add(ot[:], ot[:], b2b_sb[:])
                nc.sync.dma_start(
                    out_ext.ap()[tc_ * 128:(tc_ + 1) * 128, :], ot[:])

            for cm in (ps_m2_cm, s6_o_cm, s6_w2_cm, resid_pool_cm):
                cm.__exit__(None, None, None)

    nc.compile()
    _CACHE["nc"] = nc
    return nc


def shard_rows(c):
    """Global token rows owned by core c (four strided pieces of 128)."""
    return np.concatenate(
        [np.arange(q * 1024 + c * 128, q * 1024 + (c + 1) * 128)
         for q in range(4)])


def make_in_maps(x, ln1_g, ln1_b, w_qkv, w_out, ln2_g, ln2_b, w1, b1, w2, b2):
    import ml_dtypes
    bf16 = ml_dtypes.bfloat16
    fp8 = ml_dtypes.float8_e4m3
    x = np.asarray(x, np.float32)
    xf = np.ascontiguousarray(x.reshape(T, D))
    xt = np.ascontiguousarray(xf.T.astype(fp8))
    w_qkv_eff = np.asarray(w_qkv) * np.asarray(ln1_g)[:, None]
    bias_qkv = np.asarray(ln1_b) @ np.asarray(w_qkv)
    w1_eff = np.asarray(w1) * np.asarray(ln2_g)[:, None]
    bias_h1 = np.asarray(ln2_b) @ np.asarray(w1) + np.asarray(b1)
    w1b = np.ascontiguousarray(w1_eff.astype(bf16))
    b1gb = np.ascontiguousarray(bias_h1, np.float32).reshape(-1, 1)
    w2b = np.ascontiguousarray(np.asarray(w2).astype(bf16))
    b2b = np.tile(np.asarray(b2).astype(bf16)[None, :], (128, 1))
    km = np.arange(128)[:, None]
    qm = np.arange(512)[None, :]
    masks = np.stack([np.where(km + 128 * m <= qm, 0.0, -448.0).astype(fp8)
                      for m in range(4)])
    in_maps = []
    for c in range(NCORES):
        cs = slice(c * DLOC, (c + 1) * DLOC)
        wq = np.concatenate(
            [w_qkv_eff[:, cs], w_qkv_eff[:, D:][:, cs],
             w_qkv_eff[:, 2 * D:][:, cs]], axis=1)
        wq8 = (wq * WSC).astype(fp8)          # [D, 384] scaled fp8
        # SBUF layout [r, p, m, i, c] = wq8[p*256 + i*128 + r, m*128 + c]
        wq8_t = np.ascontiguousarray(
            wq8.reshape(ND // 2, 2, 128, 3, 128).transpose(2, 0, 3, 1, 4))
        bq = np.concatenate(
            [bias_qkv[cs], bias_qkv[D:][cs], bias_qkv[2 * D:][cs]])
        rows = shard_rows(c)
        in_maps.append({
            "xt": xt,
            "xsh": np.ascontiguousarray(xf[rows]),
            "wqkv": wq8_t,
            "nws": np.ascontiguousarray(
                (-(wq8.astype(np.float32) / WSC).sum(axis=0)).astype(
                    np.float32)).reshape(-1, 1),
            "bqkv": np.ascontiguousarray(bq, np.float32).reshape(-1, 1),
            "wout": np.ascontiguousarray(
                np.asarray(w_out)[cs].astype(bf16)),
            "w1": w1b, "b1g": b1gb, "w2": w2b,
            "b2b": b2b,
            "masks": masks,
        })
    return in_maps


def kernel(**inputs):
    nc = build()
    in_maps = make_in_maps(**inputs)
    res = bass_utils.run_bass_kernel_spmd(
        nc, in_maps, core_ids=list(range(NCORES)))
    out = np.empty((T, D), np.float32)
    for c in range(NCORES):
        out[shard_rows(c)] = res.results[c]["out"]
    return out.reshape(B, L, D).astype(np.float32)



# revision 60
# speedup vs baseline: 1.0275x; 1.0275x over previous
"""Trainium2 8-core kernel for a dense pre-norm transformer block.

Reference: h=LN1(x); qkv=h@w_qkv; causal MHA (16 heads, Dh=64);
x+=o@w_out; h2=LN2(x); x+=gelu(h2@w1+b1)@w2+b2.

Sharding (Megatron TP-8 + sequence-parallel residual):
  - heads 2c,2c+1 on core c (w_qkv column-shard, w_out row-shard)
  - MLP hidden 512 per core (w1 column-shard, w2 row-shard)
  - residual stream token-sharded: core c owns the four strided pieces
    {q*1024 + c*128 .. +128}, q=0..3. RS1 runs as four quarter-sized
    ReduceScatters fired as soon as the out-projection of each quarter
    is done (the first two hide under attention of batch 1); the
    AllGather runs per half; RS2 runs as four quarters, the first three
    hidden under remaining MLP work.
  - LN1 stats (mean/rstd) are computed shard-locally with bn_stats and
    exchanged via a tiny AllGather at kernel start; LN gains/biases are
    folded into w_qkv/w1 host-side; LN1 mean-subtraction is folded into
    the qkv matmul as a rank-1 (-colsum(w) x mean) accumulation and the
    rstd scaling is applied to the matmul output.

Compute dtype: bf16 operands, fp32 PSUM accumulation, fp32 residual.
Attention scores are computed transposed ST=[k_pos, q_pos]; both heads
run concurrently in the PE array via tile_position row-packing; softmax
denominator comes from a ones-column appended to V; causality via 4
static [128,512] masks.
"""
import numpy as np

import concourse.bass as bass
import concourse.mybir as mybir
import concourse.tile as tile
from concourse import bacc
from concourse import bass_utils
from concourse.masks import make_identity

F32 = mybir.dt.float32
BF = mybir.dt.bfloat16
F8 = mybir.dt.float8e4
AF = mybir.ActivationFunctionType
DR = mybir.MatmulPerfMode.DoubleRow
WSC = 32.0  # fp8 weight prescale (avoids e4m3 subnormals for 0.02-scale w)

NCORES = 8
B, L, D = 2, 2048, 1024
T = B * L              # 4096 tokens
TSH = T // NCORES      # 512 tokens per core (4 pieces of 128)
DH = 64                # head dim
HL = 2                 # heads per core
DLOC = HL * DH         # 128 local head features
MLPH = 4096 // NCORES  # 512 local hidden
LN_EPS = 1e-5
NT = T // 512          # 8 token tiles of 512
ND = D // 128          # 8 feature chunks
QT = L // 512          # 4 q-tiles per batch

_CACHE = {}


def build():
    if "nc" in _CACHE:
        return _CACHE["nc"]
    nc = bacc.Bacc("TRN2", target_bir_lowering=False, debug=False,
                   num_devices=NCORES)

    xt_in = nc.dram_tensor("xt", [D, T], F8, kind="ExternalInput")
    xsh_in = nc.dram_tensor("xsh", [TSH, D], F32, kind="ExternalInput")
    xshb_in = nc.dram_tensor("xshb", [TSH, D], BF, kind="ExternalInput")
    wqkv_in = nc.dram_tensor("wqkv", [128, ND // 2, 3, 2, 128], F8,
                             kind="ExternalInput")
    nws_in = nc.dram_tensor("nws", [3 * DLOC, 1], F32, kind="ExternalInput")
    bqkv_in = nc.dram_tensor("bqkv", [3 * DLOC, 1], F32, kind="ExternalInput")
    wout_in = nc.dram_tensor("wout", [DLOC, D], BF, kind="ExternalInput")
    w1_in = nc.dram_tensor("w1", [D, 4 * D], BF, kind="ExternalInput")
    b1g_in = nc.dram_tensor("b1g", [4 * D, 1], F32, kind="ExternalInput")
    w2_in = nc.dram_tensor("w2", [4 * D, D], BF, kind="ExternalInput")
    b2b_in = nc.dram_tensor("b2b", [128, D], BF, kind="ExternalInput")
    masks_in = nc.dram_tensor("masks", [4, 128, 512], F8, kind="ExternalInput")
    out_ext = nc.dram_tensor("out", [TSH, D], F32, kind="ExternalOutput")

    rg = [list(range(NCORES))]

    with tile.TileContext(nc) as tc:
        with (
            tc.tile_pool(name="const", bufs=1) as const,
            tc.tile_pool(name="wpool", bufs=1) as wpool,
            tc.tile_pool(name="dram", bufs=1, space="DRAM") as dram,
        ):
            # ---- DRAM scratch for collectives ----
            st_ag_in = dram.tile([8, 128], BF)       # (piece q, mean/rstd)
            st_ag_out = dram.tile([64, 128], BF, addr_space="Shared")
            rs1_in = [dram.tile([1024, D], BF, name=f"rs1_in{q}")
                      for q in range(4)]
            rs1_out = [dram.tile([128, D], BF, name=f"rs1_out{q}")
                       for q in range(4)]

            warm_in = dram.tile([8, 16], BF)
            warm_out = dram.tile([64, 16], BF, addr_space="Shared")

            # ---- constants ----
            ident = const.tile([128, 128], F32)
            make_identity(nc, ident[:])
            ident_bf = const.tile([128, 128], BF)
            make_identity(nc, ident_bf[:])
            ones_row = const.tile([1, 128], BF)
            nc.vector.memset(ones_row[:], 1.0)
            sc_row = const.tile([1, 128], BF)
            nc.vector.memset(sc_row[:], 1.0 / WSC)
            eps128 = const.tile([128, 1], F32)
            nc.vector.memset(eps128[:], LN_EPS)
            masks_sb = [const.tile([128, 512], F8, name=f"mask{m}")
                        for m in range(4)]

            def load_masks():
                for m in range(4):
                    nc.sync.dma_start(masks_sb[m][:], masks_in.ap()[m])

            # ---- weights resident in SBUF ----
            # wqkv_sb[r, p, m, i, c] = WSC * w_eff[p*256 + i*128 + r,
            #                                      m*128 + c], fp8
            # (pair k-tiles contiguous for dual-fp8 ldweights)
            wqkv_sb = wpool.tile([128, ND // 2, 3, 2, 128], F8, name="wqkv8")
            nc.sync.dma_start(wqkv_sb[:], wqkv_in.ap())
            nws_sb = []
            for m in range(3):
                nt_ = wpool.tile([128, 1], F32, name=f"nws{m}")
                nc.sync.dma_start(nt_[:],
                                  nws_in.ap()[m * 128:(m + 1) * 128, :])
                nws_sb.append(nt_)
            bqkv_sb = []
            for m in range(3):
                bt = wpool.tile([128, 1], F32, name=f"bqkv{m}")
                nc.sync.dma_start(bt[:], bqkv_in.ap()[m * 128:(m + 1) * 128, :])
                bqkv_sb.append(bt)
            NM = 4 * D // 128       # 32 hidden chunks (full MLP per core)
            wout_sb = wpool.tile([DLOC, D], BF)
            b2b_sb = wpool.tile([128, D], BF, name="b2b")

            def load_late_weights():
                nc.sync.dma_start(wout_sb[:], wout_in.ap())
                nc.sync.dma_start(b2b_sb[:], b2b_in.ap())
                for q in range(4):
                    nc.sync.dma_start(
                        xsv[:, q, :], xsh_in.ap()[q * 128:(q + 1) * 128, :])

            resid_pool_cm = tc.tile_pool(name="resid", bufs=1)
            resid_pool = resid_pool_cm.__enter__()
            xsv = resid_pool.tile([128, 4, D], F32)   # my shard of x
            x2_sb = resid_pool.tile([128, 4, D], F32)
            h2T = resid_pool.tile([128, ND, 512], BF)     # LN2(x2)^T shard
            g1_sb = resid_pool.tile([128, NM, 512], BF)   # gelu acts [h, tok]

            # ========== stage 0: shard-local LN1 stats + tiny AG ==========
            s0x_cm = tc.tile_pool(name="s0x", bufs=1)
            s0x = s0x_cm.__enter__()
            xsb = s0x.tile([128, 4, D], BF)    # bf16 x copy for LN1 stats
            s0_cm = tc.tile_pool(name="s0", bufs=2)
            s0 = s0_cm.__enter__()
            ps0_cm = tc.tile_pool(name="ps0", bufs=2, space="PSUM")
            ps0 = ps0_cm.__enter__()
            # absorb first-collective init latency with a no-dep dummy
            wtile = s0.tile([8, 16], BF, tag="wtile", name="wtile")
            nc.vector.memset(wtile[:], 0.0)
            nc.scalar.dma_start(warm_in[:], wtile[:])
            nc.gpsimd.collective_compute(
                "AllGather", mybir.AluOpType.bypass, replica_groups=rg,
                ins=[warm_in[:].opt()], outs=[warm_out[:].opt()])
            for q in range(4):
                nc.gpsimd.dma_start(
                    xsb[:, q, :], xshb_in.ap()[q * 128:(q + 1) * 128, :])
                stats = s0.tile([128, 2, 6], F32, tag="stats", name="stats")
                xv = xsb[:, q, :].rearrange("p (s f) -> p s f", s=2)
                for s in range(2):
                    nc.vector.bn_stats(stats[:, s, :], xv[:, s, :])
                mv = s0.tile([128, 2], F32, tag="mv", name="mv")
                nc.vector.bn_aggr(mv[:], stats[:])
                rstd0 = s0.tile([128, 1], F32, tag="rstd0", name="rstd0")
                nc.scalar.activation(rstd0[:], mv[:, 1:2], AF.Sqrt,
                                     bias=eps128[:])
                nc.vector.reciprocal_approx_fast(rstd0[:], rstd0[:])
                st2 = s0.tile([128, 2], BF, tag="st2", name="st2")
                nc.vector.tensor_copy(st2[:, 0:1], mv[:, 0:1])
                nc.vector.tensor_copy(st2[:, 1:2], rstd0[:])
                stp = ps0.tile([2, 128], BF, tag="stp", name="stp")
                nc.tensor.transpose(stp[:], st2[:], ident_bf[:])
                sts = s0.tile([2, 128], BF, tag="sts", name="sts")
                nc.vector.tensor_copy(sts[:], stp[:])
                nc.scalar.dma_start(st_ag_in[2 * q:2 * q + 2, :], sts[:])
            nc.gpsimd.collective_compute(
                "AllGather", mybir.AluOpType.bypass, replica_groups=rg,
                ins=[st_ag_in[:].opt()], outs=[st_ag_out[:].opt()])
            ps0_cm.__exit__(None, None, None)
            s0_cm.__exit__(None, None, None)
            s0x_cm.__exit__(None, None, None)

            # st_ag_out rows: c*8 + q*2 + {0:mean, 1:rstd}
            st_view = st_ag_out[:].rearrange("(c x) f -> c x f", x=8)

            praws = {}

            def do_s1_mm(tt):
                q4, h4 = tt // 2, tt % 2
                xts = s1_x.tile([128, ND, 512], F8, tag="xts")
                for d in range(ND):
                    eng = (nc.sync, nc.scalar, nc.gpsimd)[d % 3]
                    eng.dma_start(
                        xts[:, d, :],
                        xt_in.ap()[d * 128:(d + 1) * 128,
                                   tt * 512:(tt + 1) * 512])
                praws[tt] = (None, None, [])
                for m in range(3):
                    ps_q = ps_qkv.tile([128, 512], F32, tag="ps_q",
                                       name="ps_q")
                    for p in range(ND // 2):
                        nc.tensor.matmul(
                            ps_q[:],
                            wqkv_sb[:, p, m, :, :],
                            xts[:, 2 * p:2 * p + 2, :],
                            start=(p == 0), stop=(p == ND // 2 - 1),
                            perf_mode=DR)
                    praw = praw_pool.tile([128, 512], BF, tag="praw",
                                          name="praw")
                    nc.scalar.copy(praw[:], ps_q[:])
                    praws[tt][2].append(praw)

            def do_s1_fin(tt):
                q4, h4 = tt // 2, tt % 2
                _, _, praw3 = praws[tt]
                mean_bf = s1_stat.tile([1, 4, 128], BF, tag=f"mean_bf{tt}",
                                       name=f"mean_bf{tt}")
                rstd_bf = s1_stat.tile([1, 4, 128], BF, tag=f"rstd_bf{tt}",
                                       name=f"rstd_bf{tt}")
                nc.gpsimd.dma_start(
                    mean_bf[:], st_view[4 * h4:4 * h4 + 4, 2 * q4, :])
                nc.gpsimd.dma_start(
                    rstd_bf[:], st_view[4 * h4:4 * h4 + 4, 2 * q4 + 1, :])
                mean_v = mean_bf[:].rearrange("p a f -> p (a f)")
                rstd_v = rstd_bf[:].rearrange("p a f -> p (a f)")
                mr = s1_stat.tile([1, 512], BF, tag="mr", name="mr")
                nc.vector.tensor_mul(mr[:], mean_v, rstd_v)
                rstd_b = ps_st.tile([128, 512], F32, tag="pst",
                                    name="rstd_b")
                nc.tensor.matmul(rstd_b[:], sc_row[:], rstd_v,
                                 start=True, stop=True)
                rstd_bc = s1_tmp.tile([128, 512], BF, tag="rstd_bc")
                if tt >= 4:
                    nc.scalar.copy(rstd_bc[:], rstd_b[:])
                else:
                    nc.vector.tensor_copy(rstd_bc[:], rstd_b[:])
                mr_b = ps_st.tile([128, 512], F32, tag="pst", name="mr_b")
                nc.tensor.matmul(mr_b[:], ones_row[:], mr[:],
                                 start=True, stop=True)
                for m in range(3):
                    u = s1_tmp.tile([128, 512], BF, tag="pre", name="u")
                    nc.vector.tensor_mul(u[:], praw3[m][:], rstd_bc[:])
                    pre = s1_tmp.tile([128, 512], BF, tag="pre2",
                                      name="pre2")
                    nc.vector.scalar_tensor_tensor(
                        out=pre[:], in0=mr_b[:], scalar=nws_sb[m][:],
                        in1=u[:], op0=mybir.AluOpType.mult,
                        op1=mybir.AluOpType.add)
                    nc.vector.tensor_scalar(
                        out=qkvT[m][:, tt * 512:(tt + 1) * 512], in0=pre[:],
                        scalar1=bqkv_sb[m][:], scalar2=None,
                        op0=mybir.AluOpType.add)
                del praws[tt]
            # ============ stage 2/3/4 pools ============
            s2_vaug_cm = tc.tile_pool(name="s2_vaug", bufs=1)
            s2_vaug = s2_vaug_cm.__enter__()
            s2_exp_cm = tc.tile_pool(name="s2_exp", bufs=2)
            s2_exp = s2_exp_cm.__enter__()
            s2_misc_cm = tc.tile_pool(name="s2_misc", bufs=1)
            s2_misc = s2_misc_cm.__enter__()
            s3_r1_cm = tc.tile_pool(name="s3_r1", bufs=2)
            s3_r1 = s3_r1_cm.__enter__()
            s4_t_cm = tc.tile_pool(name="s4_t", bufs=1)
            s4_t = s4_t_cm.__enter__()
            ps_st_cm = tc.tile_pool(name="ps_st", bufs=3, space="PSUM")
            ps_st = ps_st_cm.__enter__()
            ps_o_cm = tc.tile_pool(name="ps_o", bufs=1, space="PSUM")
            ps_o = ps_o_cm.__enter__()
            ps_vt_cm = tc.tile_pool(name="ps_vt", bufs=1, space="PSUM")
            ps_vt = ps_vt_cm.__enter__()

            # persistent activations
            attn_pool_cm = tc.tile_pool(name="attn", bufs=1)
            attn_pool = attn_pool_cm.__enter__()
            qkvT = []
            for m in range(3):
                t_ = attn_pool.tile([128, T], BF, name=f"qkvT{m}")
                qkvT.append(t_)
            oT = attn_pool.tile([128, T], BF)

            # ================= stage 1 pools (popped mid-kernel) ==========
            s1_x_cm = tc.tile_pool(name="s1_x", bufs=2)
            s1_x = s1_x_cm.__enter__()
            s1_tmp_cm = tc.tile_pool(name="s1_tmp", bufs=3)
            s1_tmp = s1_tmp_cm.__enter__()
            s1_stat_cm = tc.tile_pool(name="s1_stat", bufs=1)
            s1_stat = s1_stat_cm.__enter__()
            ps_qkv_cm = tc.tile_pool(name="ps_qkv", bufs=2, space="PSUM")
            ps_qkv = ps_qkv_cm.__enter__()
            praw_pool_cm = tc.tile_pool(name="s1_praw", bufs=12)
            praw_pool = praw_pool_cm.__enter__()

            vaugs = {}

            def do_vaug(b, tl):
                """V-transposes for 512-token tile tl (4 k-chunks) of batch b."""
                tok0 = b * L
                if b not in vaugs:
                    vaug = s2_vaug.tile([128, HL, L // 256, 2, DH + 16], F8,
                                        tag=f"vaug{b}", name=f"vaug{b}")
                    nc.vector.memset(vaug[:, :, :, :, DH:DH + 1], 1.0)
                    nc.vector.memset(vaug[:, :, :, :, DH + 1:DH + 16], 0.0)
                    vaugs[b] = vaug
                vaug = vaugs[b]
                for hl in range(HL):
                    hrow = hl * DH
                    vT_u = qkvT[2][hrow:hrow + DH, tok0:tok0 + L]
                    for kc in range(4 * tl, 4 * tl + 4):
                        pv = ps_vt.tile([128, DH], BF, tag="pv",
                                        name="pv")
                        nc.tensor.transpose(
                            pv[:], vT_u[:, kc * 128:(kc + 1) * 128],
                            ident_bf[hrow:hrow + DH, hrow:hrow + DH])
                        if b == 0:
                            nc.scalar.copy(
                                vaug[:, hl, kc // 2, kc % 2, 0:DH], pv[:])
                        else:
                            nc.vector.tensor_copy(
                                vaug[:, hl, kc // 2, kc % 2, 0:DH], pv[:])

            def do_attn(b, js):
                tok0 = b * L
                vaug = vaugs[b]
                for j in js:
                    nk = 4 * (j + 1)
                    po = [ps_o.tile([DH + 16, 512], F32, tag=f"po{hl}",
                                    name=f"po{hl}") for hl in range(HL)]
                    for kcp in range(nk // 2):
                        kc0 = 2 * kcp
                        dm0 = kc0 - (nk - 4)
                        col0 = 128 * dm0 if dm0 > 0 else 0
                        w = 512 - col0
                        ests = []
                        for hl in range(HL):
                            hrow = hl * DH
                            qsl = qkvT[0][hrow:hrow + DH,
                                          tok0 + j * 512 + col0:
                                          tok0 + (j + 1) * 512]
                            est = s2_exp.tile([128, 2, 512], F8,
                                              tag=f"est{hl}", name=f"est{hl}")
                            for i in range(2):
                                kc = kc0 + i
                                dm = kc - (nk - 4)
                                ksl = qkvT[1][hrow:hrow + DH,
                                              tok0 + kc * 128:
                                              tok0 + (kc + 1) * 128]
                                pst = ps_st.tile([128, 512], F32, tag="pst",
                                                 name="pst")
                                nc.tensor.matmul(pst[:, :w], ksl, qsl,
                                                 start=True, stop=True,
                                                 tile_position=(hrow, 0))
                                if dm >= 0:
                                    mw = 128 * (dm + 1) - col0
                                    nc.vector.tensor_add(
                                        pst[:, :mw], pst[:, :mw],
                                        masks_sb[dm][:, col0:col0 + mw])
                                nc.scalar.activation(est[:, i, :w],
                                                     pst[:, :w],
                                                     AF.Exp, scale=0.125)
                            ests.append(est)
                        for hl in range(HL):
                            nc.tensor.matmul(po[hl][:, col0:],
                                             vaug[:, hl, kcp, :, :],
                                             ests[hl][:, :, :w],
                                             start=(kcp == 0),
                                             stop=(kcp == nk // 2 - 1),
                                             perf_mode=DR)
                    for hl in range(HL):
                        hrow = hl * DH
                        den = s2_misc.tile([1, 512], F32, tag="den",
                                           name="den")
                        nc.vector.tensor_copy(den[:], po[hl][DH:DH + 1, :])
                        rec1 = s2_misc.tile([1, 512], F32, tag="rec1",
                                            name="rec1")
                        nc.vector.reciprocal_approx_fast(rec1[:], den[:])
                        rec1b = s2_misc.tile([1, 512], BF, tag="rec1b",
                                             name="rec1b")
                        nc.vector.tensor_copy(rec1b[:], rec1[:])
                        rec_b = ps_vt.tile([64, 512], F32, tag="pv",
                                           name="rec_b")
                        nc.tensor.matmul(rec_b[:], ones_row[0:1, 0:64],
                                         rec1b[:], start=True, stop=True)
                        rec_sb = s2_misc.tile([64, 512], BF, tag="rec_sb",
                                              name="rec_sb")
                        nc.vector.tensor_copy(rec_sb[:], rec_b[:])
                        nc.vector.tensor_mul(
                            oT[hrow:hrow + DH,
                               tok0 + j * 512:tok0 + (j + 1) * 512],
                            po[hl][0:DH, :], rec_sb[:])

            def do_oproj(q):
                """out-projection for quarter q (tokens q*1024..+1024) + RS."""
                for tch in range(8):
                    row0 = q * 1024 + tch * 128
                    r1 = s3_r1.tile([128, D], BF, tag="r1", name="r1")
                    for n in range(2):
                        pop = ps_st.tile([128, 512], F32, tag="pst",
                                         name="pop")
                        nc.tensor.matmul(pop[:], oT[:, row0:row0 + 128],
                                         wout_sb[:, n * 512:(n + 1) * 512],
                                         start=True, stop=True)
                        nc.vector.tensor_copy(
                            r1[:, n * 512:(n + 1) * 512], pop[:])
                    nc.gpsimd.dma_start(
                        rs1_in[q][tch * 128:(tch + 1) * 128, :], r1[:])
                nc.gpsimd.collective_compute(
                    "ReduceScatter", mybir.AluOpType.add, replica_groups=rg,
                    ins=[rs1_in[q][:].opt()], outs=[rs1_out[q][:].opt()])

            def do_s4(q):
                """residual + LN2 + transpose for my piece of quarter q."""
                r1s = s4_t.tile([128, D], BF, tag="r1s", name="r1s")
                nc.sync.dma_start(r1s[:], rs1_out[q][:])
                nc.vector.tensor_add(x2_sb[:, q, :], xsv[:, q, :], r1s[:])
                stats = s4_t.tile([128, 2, 6], F32, tag="stats", name="stats")
                x2v = x2_sb[:, q, :].rearrange("p (s f) -> p s f", s=2)
                for s in range(2):
                    nc.vector.bn_stats(stats[:, s, :], x2v[:, s, :])
                mv = s4_t.tile([128, 2], F32, tag="mv", name="mv")
                nc.vector.bn_aggr(mv[:], stats[:])
                rstd2 = s4_t.tile([128, 1], F32, tag="rstd2", name="rstd2")
                nc.scalar.activation(rstd2[:], mv[:, 1:2], AF.Sqrt,
                                     bias=eps128[:])
                nc.vector.reciprocal_approx_fast(rstd2[:], rstd2[:])
                h2 = s4_t.tile([128, D], F32, tag="h2", name="h2")
                nc.vector.tensor_scalar(
                    out=h2[:], in0=x2_sb[:, q, :], scalar1=mv[:, 0:1],
                    scalar2=rstd2[:], op0=mybir.AluOpType.subtract,
                    op1=mybir.AluOpType.mult)
                for d in range(ND):
                    pt = ps_vt.tile([128, 128], F32, tag="pv", name="pt")
                    nc.tensor.transpose(
                        pt[:], h2[:, d * 128:(d + 1) * 128], ident[:])
                    nc.vector.tensor_copy(h2T[:, d, q * 128:(q + 1) * 128],
                                          pt[:])

            # ---------------- pipelined schedule (front) ----------------
            for tt in range(4):
                do_s1_mm(tt)
            load_masks()
            load_late_weights()
            do_s1_fin(0)
            do_s1_mm(4)
            do_vaug(0, 0)
            do_s1_fin(1)
            do_s1_mm(5)
            do_vaug(0, 1)
            do_s1_fin(2)
            do_s1_mm(6)
            do_vaug(0, 2)
            do_s1_fin(3)
            do_s1_mm(7)
            do_vaug(0, 3)
            do_attn(0, (0, 1))
            for tt in range(4, NT):
                do_s1_fin(tt)
                do_vaug(1, tt - 4)
            do_oproj(0)
            do_attn(0, (2, 3))

            # s1 done: free its SBUF/PSUM, bring in the full w1 for DP-MLP
            praw_pool_cm.__exit__(None, None, None)
            for cm in (ps_qkv_cm, s1_stat_cm, s1_tmp_cm, s1_x_cm):
                cm.__exit__(None, None, None)
            w1p_cm = tc.tile_pool(name="w1p", bufs=1)
            w1p = w1p_cm.__enter__()
            w1_sb = [w1p.tile([128, 4 * D], BF, name=f"w1_{d}")
                     for d in range(ND)]
            b1g_sb = w1p.tile([128, NM], F32, name="b1g")
            nc.sync.dma_start(
                b1g_sb[:],
                b1g_in.ap().rearrange("(m r) o -> r (m o)", r=128))
            for d in range(ND):
                nc.gpsimd.dma_start(w1_sb[d][:],
                                    w1_in.ap()[d * 128:(d + 1) * 128, :])
            ps_m1_cm = tc.tile_pool(name="ps_m1", bufs=2, space="PSUM")
            ps_m1 = ps_m1_cm.__enter__()

            def do_mlp1(qp, ms):
                """MLP1+GELU for token half qp (256 cols), hidden chunks ms."""
                c0 = qp * 256
                for m in ms:
                    pm1 = ps_m1.tile([128, 256], F32, tag="pm1", name="pm1")
                    for d in range(ND):
                        nc.tensor.matmul(
                            pm1[:], w1_sb[d][:, m * 128:(m + 1) * 128],
                            h2T[:, d, c0:c0 + 256], start=(d == 0),
                            stop=(d == ND - 1))
                    nc.scalar.activation(g1_sb[:, m, c0:c0 + 256], pm1[:],
                                         AF.Gelu, bias=b1g_sb[:, m:m + 1])

            do_oproj(1)
            do_attn(1, (2, 3))   # hides RS1_0 + RS1_1
            do_oproj(3)
            do_s4(0)
            do_s4(1)
            do_attn(1, (0, 1))   # hides RS1_3
            do_oproj(2)
            do_mlp1(0, range(NM))      # hides RS1_2
            do_s4(3)
            do_s4(2)
            do_mlp1(1, range(NM))

            for cm in (ps_m1_cm, w1p_cm, attn_pool_cm,
                       ps_vt_cm, ps_o_cm, ps_st_cm, s4_t_cm,
                       s3_r1_cm, s2_misc_cm, s2_exp_cm, s2_vaug_cm):
                cm.__exit__(None, None, None)

            # ---- stage 6: MLP2 (m-major, all 8 PSUM banks accumulate) ----
            s6_w2_cm = tc.tile_pool(name="s6_w2", bufs=8)
            s6_w2 = s6_w2_cm.__enter__()
            s6_o_cm = tc.tile_pool(name="s6_o", bufs=2)
            s6_o = s6_o_cm.__enter__()
            ps_m2_cm = tc.tile_pool(name="ps_m2", bufs=1, space="PSUM")
            ps_m2 = ps_m2_cm.__enter__()

            pm2 = [ps_m2.tile([128, 1024], F32, tag=f"pm2_{tc_}",
                              name=f"pm2_{tc_}") for tc_ in range(4)]
            for m in range(NM):
                w2c = s6_w2.tile([128, D], BF, tag="w2c", name="w2c")
                nc.sync.dma_start(w2c[:],
                                   w2_in.ap()[m * 128:(m + 1) * 128, :])
                for tc_ in range(4):
                    for n2 in range(2):
                        nc.tensor.matmul(
                            pm2[tc_][:, n2 * 512:(n2 + 1) * 512],
                            g1_sb[:, m, tc_ * 128:(tc_ + 1) * 128],
                            w2c[:, n2 * 512:(n2 + 1) * 512],
                            start=(m == 0), stop=(m == NM - 1))
            for tc_ in range(4):
                ot = s6_o.tile([128, D], F32, tag="ot", name="ot")
                nc.vector.tensor_add(ot[:], x2_sb[:, tc_, :], pm2[tc_][:])
                nc.vector.tensor_add(ot[:], ot[:], b2b_sb[:])
                nc.sync.dma_start(
                    out_ext.ap()[tc_ * 128:(tc_ + 1) * 128, :], ot[:])

            for cm in (ps_m2_cm, s6_o_cm, s6_w2_cm, resid_pool_cm):
                cm.__exit__(None, None, None)

    nc.compile()
    _CACHE["nc"] = nc
    return nc


def shard_rows(c):
    """Global token rows owned by core c (four strided pieces of 128)."""
    return np.concatenate(
        [np.arange(q * 1024 + c * 128, q * 1024 + (c + 1) * 128)
         for q in range(4)])


def make_in_maps(x, ln1_g, ln1_b, w_qkv, w_out, ln2_g, ln2_b, w1, b1, w2, b2):
    import ml_dtypes
    bf16 = ml_dtypes.bfloat16
    fp8 = ml_dtypes.float8_e4m3
    x = np.asarray(x, np.float32)
    xf = np.ascontiguousarray(x.reshape(T, D))
    xt = np.ascontiguousarray(xf.T.astype(fp8))
    w_qkv_eff = np.asarray(w_qkv) * np.asarray(ln1_g)[:, None]
    bias_qkv = np.asarray(ln1_b) @ np.asarray(w_qkv)
    w1_eff = np.asarray(w1) * np.asarray(ln2_g)[:, None]
    bias_h1 = np.asarray(ln2_b) @ np.asarray(w1) + np.asarray(b1)
    w1b = np.ascontiguousarray(w1_eff.astype(bf16))
    b1gb = np.ascontiguousarray(bias_h1, np.float32).reshape(-1, 1)
    w2b = np.ascontiguousarray(np.asarray(w2).astype(bf16))
    b2b = np.tile(np.asarray(b2).astype(bf16)[None, :], (128, 1))
    km = np.arange(128)[:, None]
    qm = np.arange(512)[None, :]
    masks = np.stack([np.where(km + 128 * m <= qm, 0.0, -448.0).astype(fp8)
                      for m in range(4)])
    in_maps = []
    for c in range(NCORES):
        cs = slice(c * DLOC, (c + 1) * DLOC)
        wq = np.concatenate(
            [w_qkv_eff[:, cs], w_qkv_eff[:, D:][:, cs],
             w_qkv_eff[:, 2 * D:][:, cs]], axis=1)
        wq8 = (wq * WSC).astype(fp8)          # [D, 384] scaled fp8
        # SBUF layout [r, p, m, i, c] = wq8[p*256 + i*128 + r, m*128 + c]
        wq8_t = np.ascontiguousarray(
            wq8.reshape(ND // 2, 2, 128, 3, 128).transpose(2, 0, 3, 1, 4))
        bq = np.concatenate(
            [bias_qkv[cs], bias_qkv[D:][cs], bias_qkv[2 * D:][cs]])
        rows = shard_rows(c)
        in_maps.append({
            "xt": xt,
            "xsh": np.ascontiguousarray(xf[rows]),
            "xshb": np.ascontiguousarray(xf[rows].astype(bf16)),
            "wqkv": wq8_t,
            "nws": np.ascontiguousarray(
                (-(wq8.astype(np.float32) / WSC).sum(axis=0)).astype(
                    np.float32)).reshape(-1, 1),
            "bqkv": np.ascontiguousarray(bq, np.float32).reshape(-1, 1),
            "wout": np.ascontiguousarray(
                np.asarray(w_out)[cs].astype(bf16)),
            "w1": w1b, "b1g": b1gb, "w2": w2b,
            "b2b": b2b,
            "masks": masks,
        })
    return in_maps


def kernel(**inputs):
    nc = build()
    in_maps = make_in_maps(**inputs)
    res = bass_utils.run_bass_kernel_spmd(
        nc, in_maps, core_ids=list(range(NCORES)))
    out = np.empty((T, D), np.float32)
    for c in range(NCORES):
        out[shard_rows(c)] = res.results[c]["out"]
    return out.reshape(B, L, D).astype(np.float32)



# revision 65
# speedup vs baseline: 1.0442x; 1.0163x over previous
"""Trainium2 8-core kernel for a dense pre-norm transformer block.

Reference: h=LN1(x); qkv=h@w_qkv; causal MHA (16 heads, Dh=64);
x+=o@w_out; h2=LN2(x); x+=gelu(h2@w1+b1)@w2+b2.

Sharding (Megatron TP attention + data-parallel MLP):
  - heads 2c,2c+1 on core c (w_qkv column-shard, w_out row-shard);
    the attention out-projection partials are combined with four
    quarter-sized ReduceScatters (RS1), each fired as soon as its
    token quarter's out-projection finishes and hidden under the
    remaining attention / MLP1 compute.
  - residual stream token-sharded: core c owns the four strided
    pieces {q*1024 + c*128 .. +128}, q=0..3.
  - the MLP runs DATA-parallel: every core holds the full w1/w2 and
    computes the whole MLP for its own 512 tokens, so there is no
    AllGather and no second ReduceScatter; w1 (8MB) is DMA'd during
    attention into the SBUF freed by stage-1 pools, w2 streams
    through an 8-chunk ring during MLP2.  MLP2 accumulates m-major
    into all 8 PSUM banks (one [128,1024] fp32 tile per token piece).
  - LN1 stats (mean/rstd) are computed shard-locally with bn_stats
    (from a bf16 copy of x) and exchanged via a tiny AllGather at
    kernel start (preceded by a no-dep dummy AllGather that absorbs
    first-collective init/skew latency); LN gains/biases are folded
    into w_qkv/w1 host-side; LN1 mean-subtraction is folded into the
    qkv matmul as a rank-1 (-colsum(w) x mean) accumulation and the
    rstd scaling is applied to the matmul output.

Compute dtypes: the qkv matmul and the attention*V matmul run in fp8
(e4m3) with MatmulPerfMode.DoubleRow (256-deep contraction per pass,
2x PE throughput); weights are prescaled by 32 to dodge e4m3
subnormals and the scale is folded back via the rstd broadcast.
Scores stay bf16 (K=64, no DoubleRow win); the out-projection and
both MLP matmuls stay bf16 for accuracy.  PSUM accumulates fp32 and
the residual stream is fp32.

Attention scores are computed transposed ST=[k_pos, q_pos]; the two
heads run concurrently in the PE array via tile_position row-packing;
softmax denominator comes from a ones-column appended to V (vaug
padded to 80 cols so dual-fp8 LDWEIGHTS step%16==0 holds); causality
via additive -448 masks applied to the diagonal PSUM band before the
exp, which writes fp8 directly.
"""
import numpy as np

import concourse.bass as bass
import concourse.mybir as mybir
import concourse.tile as tile
from concourse import bacc
from concourse import bass_utils
from concourse.masks import make_identity

F32 = mybir.dt.float32
BF = mybir.dt.bfloat16
F8 = mybir.dt.float8e4
AF = mybir.ActivationFunctionType
DR = mybir.MatmulPerfMode.DoubleRow
WSC = 32.0  # fp8 weight prescale (avoids e4m3 subnormals for 0.02-scale w)

NCORES = 8
B, L, D = 2, 2048, 1024
T = B * L              # 4096 tokens
TSH = T // NCORES      # 512 tokens per core (4 pieces of 128)
DH = 64                # head dim
HL = 2                 # heads per core
DLOC = HL * DH         # 128 local head features
MLPH = 4096 // NCORES  # 512 local hidden
LN_EPS = 1e-5
NT = T // 512          # 8 token tiles of 512
ND = D // 128          # 8 feature chunks
QT = L // 512          # 4 q-tiles per batch

_CACHE = {}


def build():
    if "nc" in _CACHE:
        return _CACHE["nc"]
    nc = bacc.Bacc("TRN2", target_bir_lowering=False, debug=False,
                   num_devices=NCORES)

    xt_in = nc.dram_tensor("xt", [D, T], F8, kind="ExternalInput")
    xsh_in = nc.dram_tensor("xsh", [TSH, D], F32, kind="ExternalInput")
    xshb_in = nc.dram_tensor("xshb", [TSH, D], BF, kind="ExternalInput")
    wqkv_in = nc.dram_tensor("wqkv", [128, ND // 2, 3, 2, 128], F8,
                             kind="ExternalInput")
    nws_in = nc.dram_tensor("nws", [3 * DLOC, 1], F32, kind="ExternalInput")
    bqkv_in = nc.dram_tensor("bqkv", [3 * DLOC, 1], F32, kind="ExternalInput")
    wout_in = nc.dram_tensor("wout", [DLOC, D], BF, kind="ExternalInput")
    w1_in = nc.dram_tensor("w1", [D, 4 * D], BF, kind="ExternalInput")
    b1g_in = nc.dram_tensor("b1g", [4 * D, 1], F32, kind="ExternalInput")
    w2_in = nc.dram_tensor("w2", [4 * D, D], BF, kind="ExternalInput")
    b2b_in = nc.dram_tensor("b2b", [128, D], BF, kind="ExternalInput")
    masks_in = nc.dram_tensor("masks", [4, 128, 512], F8, kind="ExternalInput")
    out_ext = nc.dram_tensor("out", [TSH, D], F32, kind="ExternalOutput")

    rg = [list(range(NCORES))]

    with tile.TileContext(nc) as tc:
        with (
            tc.tile_pool(name="const", bufs=1) as const,
            tc.tile_pool(name="wpool", bufs=1) as wpool,
            tc.tile_pool(name="dram", bufs=1, space="DRAM") as dram,
        ):
            # ---- DRAM scratch for collectives ----
            st_ag_in = dram.tile([8, 128], BF)       # (piece q, mean/rstd)
            st_ag_out = dram.tile([64, 128], BF, addr_space="Shared")
            rs1_in = [dram.tile([1024, D], BF, name=f"rs1_in{q}")
                      for q in range(4)]
            rs1_out = [dram.tile([128, D], BF, name=f"rs1_out{q}")
                       for q in range(4)]

            warm_in = dram.tile([8, 16], BF)
            warm_out = dram.tile([64, 16], BF, addr_space="Shared")

            # ---- constants ----
            ident = const.tile([128, 128], F32)
            make_identity(nc, ident[:])
            ident_bf = const.tile([128, 128], BF)
            make_identity(nc, ident_bf[:])
            ones_row = const.tile([1, 128], BF)
            nc.vector.memset(ones_row[:], 1.0)
            sc_row = const.tile([1, 128], BF)
            nc.vector.memset(sc_row[:], 1.0 / WSC)
            eps128 = const.tile([128, 1], F32)
            nc.vector.memset(eps128[:], LN_EPS)
            masks_sb = [const.tile([128, 512], F8, name=f"mask{m}")
                        for m in range(4)]

            def load_masks():
                for m in range(4):
                    nc.sync.dma_start(masks_sb[m][:], masks_in.ap()[m])

            # ---- weights resident in SBUF ----
            # wqkv_sb[r, p, m, i, c] = WSC * w_eff[p*256 + i*128 + r,
            #                                      m*128 + c], fp8
            # (pair k-tiles contiguous for dual-fp8 ldweights)
            wqkv_sb = wpool.tile([128, ND // 2, 3, 2, 128], F8, name="wqkv8")
            nc.sync.dma_start(wqkv_sb[:], wqkv_in.ap())
            nws_sb = []
            for m in range(3):
                nt_ = wpool.tile([128, 1], F32, name=f"nws{m}")
                nc.sync.dma_start(nt_[:],
                                  nws_in.ap()[m * 128:(m + 1) * 128, :])
                nws_sb.append(nt_)
            bqkv_sb = []
            for m in range(3):
                bt = wpool.tile([128, 1], F32, name=f"bqkv{m}")
                nc.sync.dma_start(bt[:], bqkv_in.ap()[m * 128:(m + 1) * 128, :])
                bqkv_sb.append(bt)
            NM = 4 * D // 128       # 32 hidden chunks (full MLP per core)
            wout_sb = wpool.tile([DLOC, D], BF)
            b2b_sb = wpool.tile([128, D], BF, name="b2b")

            def load_late_weights():
                nc.sync.dma_start(wout_sb[:], wout_in.ap())
                nc.sync.dma_start(b2b_sb[:], b2b_in.ap())
                for q in range(4):
                    nc.sync.dma_start(
                        xsv[:, q, :], xsh_in.ap()[q * 128:(q + 1) * 128, :])

            resid_pool_cm = tc.tile_pool(name="resid", bufs=1)
            resid_pool = resid_pool_cm.__enter__()
            xsv = resid_pool.tile([128, 4, D], F32)   # my shard of x
            x2_sb = resid_pool.tile([128, 4, D], F32)
            h2T = resid_pool.tile([128, ND, 512], BF)     # LN2(x2)^T shard
            g1_sb = resid_pool.tile([128, NM, 512], BF)   # gelu acts [h, tok]

            # ========== stage 0: shard-local LN1 stats + tiny AG ==========
            s0x_cm = tc.tile_pool(name="s0x", bufs=1)
            s0x = s0x_cm.__enter__()
            xsb = s0x.tile([128, 4, D], BF)    # bf16 x copy for LN1 stats
            s0_cm = tc.tile_pool(name="s0", bufs=2)
            s0 = s0_cm.__enter__()
            ps0_cm = tc.tile_pool(name="ps0", bufs=2, space="PSUM")
            ps0 = ps0_cm.__enter__()
            # absorb first-collective init latency with a no-dep dummy
            wtile = s0.tile([8, 16], BF, tag="wtile", name="wtile")
            nc.vector.memset(wtile[:], 0.0)
            nc.scalar.dma_start(warm_in[:], wtile[:])
            nc.gpsimd.collective_compute(
                "AllGather", mybir.AluOpType.bypass, replica_groups=rg,
                ins=[warm_in[:].opt()], outs=[warm_out[:].opt()])
            for q in range(4):
                nc.gpsimd.dma_start(
                    xsb[:, q, :], xshb_in.ap()[q * 128:(q + 1) * 128, :])
                stats = s0.tile([128, 2, 6], F32, tag="stats", name="stats")
                xv = xsb[:, q, :].rearrange("p (s f) -> p s f", s=2)
                for s in range(2):
                    nc.vector.bn_stats(stats[:, s, :], xv[:, s, :])
                mv = s0.tile([128, 2], F32, tag="mv", name="mv")
                nc.vector.bn_aggr(mv[:], stats[:])
                rstd0 = s0.tile([128, 1], F32, tag="rstd0", name="rstd0")
                nc.scalar.activation(rstd0[:], mv[:, 1:2], AF.Sqrt,
                                     bias=eps128[:])
                nc.vector.reciprocal_approx_fast(rstd0[:], rstd0[:])
                st2 = s0.tile([128, 2], BF, tag="st2", name="st2")
                nc.vector.tensor_copy(st2[:, 0:1], mv[:, 0:1])
                nc.vector.tensor_copy(st2[:, 1:2], rstd0[:])
                stp = ps0.tile([2, 128], BF, tag="stp", name="stp")
                nc.tensor.transpose(stp[:], st2[:], ident_bf[:])
                sts = s0.tile([2, 128], BF, tag="sts", name="sts")
                nc.vector.tensor_copy(sts[:], stp[:])
                nc.scalar.dma_start(st_ag_in[2 * q:2 * q + 2, :], sts[:])
            nc.gpsimd.collective_compute(
                "AllGather", mybir.AluOpType.bypass, replica_groups=rg,
                ins=[st_ag_in[:].opt()], outs=[st_ag_out[:].opt()])
            ps0_cm.__exit__(None, None, None)
            s0_cm.__exit__(None, None, None)
            s0x_cm.__exit__(None, None, None)

            # st_ag_out rows: c*8 + q*2 + {0:mean, 1:rstd}
            st_view = st_ag_out[:].rearrange("(c x) f -> c x f", x=8)

            praws = {}

            def do_s1_mm(tt):
                q4, h4 = tt // 2, tt % 2
                xts = s1_x.tile([128, ND, 512], F8, tag="xts")
                eng = (nc.sync, nc.scalar)[tt % 2]
                eng.dma_start(
                    xts[:],
                    xt_in.ap()[:, tt * 512:(tt + 1) * 512].rearrange(
                        "(c p) t -> p c t", p=128))
                praws[tt] = (None, None, [])
                for m in range(3):
                    ps_q = ps_qkv.tile([128, 512], F32, tag="ps_q",
                                       name="ps_q")
                    for p in range(ND // 2):
                        nc.tensor.matmul(
                            ps_q[:],
                            wqkv_sb[:, p, m, :, :],
                            xts[:, 2 * p:2 * p + 2, :],
                            start=(p == 0), stop=(p == ND // 2 - 1),
                            perf_mode=DR)
                    praw = praw_pool.tile([128, 512], BF, tag="praw",
                                          name="praw")
                    nc.scalar.copy(praw[:], ps_q[:])
                    praws[tt][2].append(praw)

            def do_s1_fin(tt):
                q4, h4 = tt // 2, tt % 2
                _, _, praw3 = praws[tt]
                mean_bf = s1_stat.tile([1, 4, 128], BF, tag=f"mean_bf{tt}",
                                       name=f"mean_bf{tt}")
                rstd_bf = s1_stat.tile([1, 4, 128], BF, tag=f"rstd_bf{tt}",
                                       name=f"rstd_bf{tt}")
                nc.gpsimd.dma_start(
                    mean_bf[:], st_view[4 * h4:4 * h4 + 4, 2 * q4, :])
                nc.gpsimd.dma_start(
                    rstd_bf[:], st_view[4 * h4:4 * h4 + 4, 2 * q4 + 1, :])
                mean_v = mean_bf[:].rearrange("p a f -> p (a f)")
                rstd_v = rstd_bf[:].rearrange("p a f -> p (a f)")
                mr = s1_stat.tile([1, 512], BF, tag="mr", name="mr")
                nc.vector.tensor_mul(mr[:], mean_v, rstd_v)
                rstd_b = ps_st.tile([128, 512], F32, tag="pst",
                                    name="rstd_b")
                nc.tensor.matmul(rstd_b[:], sc_row[:], rstd_v,
                                 start=True, stop=True)
                rstd_bc = s1_tmp.tile([128, 512], BF, tag="rstd_bc")
                if tt >= 4:
                    nc.scalar.copy(rstd_bc[:], rstd_b[:])
                else:
                    nc.vector.tensor_copy(rstd_bc[:], rstd_b[:])
                mr_b = ps_st.tile([128, 512], F32, tag="pst", name="mr_b")
                nc.tensor.matmul(mr_b[:], ones_row[:], mr[:],
                                 start=True, stop=True)
                for m in range(3):
                    u = s1_tmp.tile([128, 512], BF, tag="pre", name="u")
                    nc.vector.tensor_mul(u[:], praw3[m][:], rstd_bc[:])
                    pre = s1_tmp.tile([128, 512], BF, tag="pre2",
                                      name="pre2")
                    nc.vector.scalar_tensor_tensor(
                        out=pre[:], in0=mr_b[:], scalar=nws_sb[m][:],
                        in1=u[:], op0=mybir.AluOpType.mult,
                        op1=mybir.AluOpType.add)
                    nc.vector.tensor_scalar(
                        out=qkvT[m][:, tt * 512:(tt + 1) * 512], in0=pre[:],
                        scalar1=bqkv_sb[m][:], scalar2=None,
                        op0=mybir.AluOpType.add)
                del praws[tt]
            # ============ stage 2/3/4 pools ============
            s2_vaug_cm = tc.tile_pool(name="s2_vaug", bufs=1)
            s2_vaug = s2_vaug_cm.__enter__()
            s2_exp_cm = tc.tile_pool(name="s2_exp", bufs=2)
            s2_exp = s2_exp_cm.__enter__()
            s2_misc_cm = tc.tile_pool(name="s2_misc", bufs=1)
            s2_misc = s2_misc_cm.__enter__()
            s3_r1_cm = tc.tile_pool(name="s3_r1", bufs=2)
            s3_r1 = s3_r1_cm.__enter__()
            s4_t_cm = tc.tile_pool(name="s4_t", bufs=1)
            s4_t = s4_t_cm.__enter__()
            ps_st_cm = tc.tile_pool(name="ps_st", bufs=3, space="PSUM")
            ps_st = ps_st_cm.__enter__()
            ps_o_cm = tc.tile_pool(name="ps_o", bufs=1, space="PSUM")
            ps_o = ps_o_cm.__enter__()
            ps_vt_cm = tc.tile_pool(name="ps_vt", bufs=1, space="PSUM")
            ps_vt = ps_vt_cm.__enter__()

            # persistent activations
            attn_pool_cm = tc.tile_pool(name="attn", bufs=1)
            attn_pool = attn_pool_cm.__enter__()
            qkvT = []
            for m in range(3):
                t_ = attn_pool.tile([128, T], BF, name=f"qkvT{m}")
                qkvT.append(t_)
            oT = attn_pool.tile([128, T], BF)

            # ================= stage 1 pools (popped mid-kernel) ==========
            s1_x_cm = tc.tile_pool(name="s1_x", bufs=2)
            s1_x = s1_x_cm.__enter__()
            s1_tmp_cm = tc.tile_pool(name="s1_tmp", bufs=3)
            s1_tmp = s1_tmp_cm.__enter__()
            s1_stat_cm = tc.tile_pool(name="s1_stat", bufs=1)
            s1_stat = s1_stat_cm.__enter__()
            ps_qkv_cm = tc.tile_pool(name="ps_qkv", bufs=2, space="PSUM")
            ps_qkv = ps_qkv_cm.__enter__()
            praw_pool_cm = tc.tile_pool(name="s1_praw", bufs=12)
            praw_pool = praw_pool_cm.__enter__()

            vaugs = {}

            def do_vaug(b, tl):
                """V-transposes for 512-token tile tl (4 k-chunks) of batch b."""
                tok0 = b * L
                if b not in vaugs:
                    vaug = s2_vaug.tile([128, HL, L // 256, 2, DH + 16], F8,
                                        tag=f"vaug{b}", name=f"vaug{b}")
                    nc.vector.memset(vaug[:, :, :, :, DH:DH + 1], 1.0)
                    nc.vector.memset(vaug[:, :, :, :, DH + 1:DH + 16], 0.0)
                    vaugs[b] = vaug
                vaug = vaugs[b]
                for hl in range(HL):
                    hrow = hl * DH
                    vT_u = qkvT[2][hrow:hrow + DH, tok0:tok0 + L]
                    for kc in range(4 * tl, 4 * tl + 4):
                        pv = ps_vt.tile([128, DH], BF, tag="pv",
                                        name="pv")
                        nc.tensor.transpose(
                            pv[:], vT_u[:, kc * 128:(kc + 1) * 128],
                            ident_bf[hrow:hrow + DH, hrow:hrow + DH])
                        if b == 0:
                            nc.scalar.copy(
                                vaug[:, hl, kc // 2, kc % 2, 0:DH], pv[:])
                        else:
                            nc.vector.tensor_copy(
                                vaug[:, hl, kc // 2, kc % 2, 0:DH], pv[:])

            def do_attn(b, js, fill=None):
                tok0 = b * L
                vaug = vaugs[b]
                for j in js:
                    nk = 4 * (j + 1)
                    po = [ps_o.tile([DH + 16, 512], F32, tag=f"po{hl}",
                                    name=f"po{hl}") for hl in range(HL)]
                    for kcp in range(nk // 2):
                        kc0 = 2 * kcp
                        dm0 = kc0 - (nk - 4)
                        col0 = 128 * dm0 if dm0 > 0 else 0
                        w = 512 - col0
                        ests = []
                        for hl in range(HL):
                            hrow = hl * DH
                            qsl = qkvT[0][hrow:hrow + DH,
                                          tok0 + j * 512 + col0:
                                          tok0 + (j + 1) * 512]
                            est = s2_exp.tile([128, 2, 512], F8,
                                              tag=f"est{hl}", name=f"est{hl}")
                            for i in range(2):
                                kc = kc0 + i
                                dm = kc - (nk - 4)
                                ksl = qkvT[1][hrow:hrow + DH,
                                              tok0 + kc * 128:
                                              tok0 + (kc + 1) * 128]
                                pst = ps_st.tile([128, 512], F32, tag="pst",
                                                 name="pst")
                                nc.tensor.matmul(pst[:, :w], ksl, qsl,
                                                 start=True, stop=True,
                                                 tile_position=(hrow, 0))
                                if dm >= 0:
                                    mw = 128 * (dm + 1) - col0
                                    nc.vector.tensor_add(
                                        pst[:, :mw], pst[:, :mw],
                                        masks_sb[dm][:, col0:col0 + mw])
                                nc.scalar.activation(est[:, i, :w],
                                                     pst[:, :w],
                                                     AF.Exp, scale=0.125)
                            ests.append(est)
                        for hl in range(HL):
                            nc.tensor.matmul(po[hl][:, col0:],
                                             vaug[:, hl, kcp, :, :],
                                             ests[hl][:, :, :w],
                                             start=(kcp == 0),
                                             stop=(kcp == nk // 2 - 1),
                                             perf_mode=DR)
                        if fill is not None:
                            fill()
                    for hl in range(HL):
                        hrow = hl * DH
                        den = s2_misc.tile([1, 512], F32, tag="den",
                                           name="den")
                        nc.vector.tensor_copy(den[:], po[hl][DH:DH + 1, :])
                        rec1 = s2_misc.tile([1, 512], F32, tag="rec1",
                                            name="rec1")
                        nc.vector.reciprocal_approx_fast(rec1[:], den[:])
                        rec1b = s2_misc.tile([1, 512], BF, tag="rec1b",
                                             name="rec1b")
                        nc.vector.tensor_copy(rec1b[:], rec1[:])
                        rec_b = ps_vt.tile([64, 512], F32, tag="pv",
                                           name="rec_b")
                        nc.tensor.matmul(rec_b[:], ones_row[0:1, 0:64],
                                         rec1b[:], start=True, stop=True)
                        rec_sb = s2_misc.tile([64, 512], BF, tag="rec_sb",
                                              name="rec_sb")
                        nc.vector.tensor_copy(rec_sb[:], rec_b[:])
                        nc.vector.tensor_mul(
                            oT[hrow:hrow + DH,
                               tok0 + j * 512:tok0 + (j + 1) * 512],
                            po[hl][0:DH, :], rec_sb[:])

            def do_oproj(q):
                """out-projection for quarter q (tokens q*1024..+1024) + RS."""
                for tch in range(8):
                    row0 = q * 1024 + tch * 128
                    r1 = s3_r1.tile([128, D], BF, tag="r1", name="r1")
                    for n in range(2):
                        pop = ps_st.tile([128, 512], F32, tag="pst",
                                         name="pop")
                        nc.tensor.matmul(pop[:], oT[:, row0:row0 + 128],
                                         wout_sb[:, n * 512:(n + 1) * 512],
                                         start=True, stop=True)
                        if n == 0:
                            nc.vector.tensor_copy(
                                r1[:, n * 512:(n + 1) * 512], pop[:])
                        else:
                            nc.scalar.copy(
                                r1[:, n * 512:(n + 1) * 512], pop[:])
                    nc.gpsimd.dma_start(
                        rs1_in[q][tch * 128:(tch + 1) * 128, :], r1[:])
                nc.gpsimd.collective_compute(
                    "ReduceScatter", mybir.AluOpType.add, replica_groups=rg,
                    ins=[rs1_in[q][:].opt()], outs=[rs1_out[q][:].opt()])

            def do_s4(q):
                """residual + LN2 + transpose for my piece of quarter q."""
                r1s = s4_t.tile([128, D], BF, tag="r1s", name="r1s")
                nc.sync.dma_start(r1s[:], rs1_out[q][:])
                nc.vector.tensor_add(x2_sb[:, q, :], xsv[:, q, :], r1s[:])
                stats = s4_t.tile([128, 2, 6], F32, tag="stats", name="stats")
                x2v = x2_sb[:, q, :].rearrange("p (s f) -> p s f", s=2)
                for s in range(2):
                    nc.vector.bn_stats(stats[:, s, :], x2v[:, s, :])
                mv = s4_t.tile([128, 2], F32, tag="mv", name="mv")
                nc.vector.bn_aggr(mv[:], stats[:])
                rstd2 = s4_t.tile([128, 1], F32, tag="rstd2", name="rstd2")
                nc.scalar.activation(rstd2[:], mv[:, 1:2], AF.Sqrt,
                                     bias=eps128[:])
                nc.vector.reciprocal_approx_fast(rstd2[:], rstd2[:])
                h2 = s4_t.tile([128, D], F32, tag="h2", name="h2")
                nc.vector.tensor_scalar(
                    out=h2[:], in0=x2_sb[:, q, :], scalar1=mv[:, 0:1],
                    scalar2=rstd2[:], op0=mybir.AluOpType.subtract,
                    op1=mybir.AluOpType.mult)
                for d in range(ND):
                    pt = ps_vt.tile([128, 128], F32, tag="pv", name="pt")
                    nc.tensor.transpose(
                        pt[:], h2[:, d * 128:(d + 1) * 128], ident[:])
                    nc.vector.tensor_copy(h2T[:, d, q * 128:(q + 1) * 128],
                                          pt[:])

            # ---------------- pipelined schedule (front) ----------------
            for tt in range(4):
                do_s1_mm(tt)
            load_masks()
            load_late_weights()
            do_s1_fin(0)
            do_s1_mm(4)
            do_vaug(0, 0)
            do_s1_fin(1)
            do_s1_mm(5)
            do_vaug(0, 1)
            do_s1_fin(2)
            do_s1_mm(6)
            do_vaug(0, 2)
            do_s1_fin(3)
            do_s1_mm(7)
            do_vaug(0, 3)
            do_attn(0, (0, 1))
            do_oproj(0)
            do_attn(0, (2, 3))
            for tt in range(4, NT):
                do_s1_fin(tt)
                do_vaug(1, tt - 4)

            # s1 done: free its SBUF/PSUM, bring in the full w1 for DP-MLP
            praw_pool_cm.__exit__(None, None, None)
            for cm in (ps_qkv_cm, s1_stat_cm, s1_tmp_cm, s1_x_cm):
                cm.__exit__(None, None, None)
            w1p_cm = tc.tile_pool(name="w1p", bufs=1)
            w1p = w1p_cm.__enter__()
            w1_sb = [w1p.tile([128, 4 * D], BF, name=f"w1_{d}")
                     for d in range(ND)]
            b1g_sb = w1p.tile([128, NM], F32, name="b1g")
            nc.sync.dma_start(
                b1g_sb[:],
                b1g_in.ap().rearrange("(m r) o -> r (m o)", r=128))
            for d in range(ND):
                nc.gpsimd.dma_start(w1_sb[d][:],
                                    w1_in.ap()[d * 128:(d + 1) * 128, :])
            ps_m1_cm = tc.tile_pool(name="ps_m1", bufs=2, space="PSUM")
            ps_m1 = ps_m1_cm.__enter__()

            def do_mlp1(qp, ms, raw=False):
                """MLP1 for token half qp (256 cols), hidden chunks ms.
                raw=True defers GELU: PSUM is copied to g1 by vector and a
                later batched gelu pass applies the activation in place
                (avoids exp<->gelu ACT-table thrash mid-attention)."""
                c0 = qp * 256
                for m in ms:
                    pm1 = ps_m1.tile([128, 256], F32, tag="pm1", name="pm1")
                    for d in range(ND):
                        nc.tensor.matmul(
                            pm1[:], w1_sb[d][:, m * 128:(m + 1) * 128],
                            h2T[:, d, c0:c0 + 256], start=(d == 0),
                            stop=(d == ND - 1))
                    if raw:
                        nc.vector.tensor_copy(g1_sb[:, m, c0:c0 + 256],
                                              pm1[:])
                    else:
                        nc.scalar.activation(g1_sb[:, m, c0:c0 + 256],
                                             pm1[:], AF.Gelu,
                                             bias=b1g_sb[:, m:m + 1])

            def do_gelu_pass(qp, ms):
                c0 = qp * 256
                for m in ms:
                    nc.scalar.activation(g1_sb[:, m, c0:c0 + 256],
                                         g1_sb[:, m, c0:c0 + 256],
                                         AF.Gelu, bias=b1g_sb[:, m:m + 1])

            do_oproj(1)
            do_attn(1, (2, 3))   # hides RS1_0 + RS1_1
            do_oproj(3)
            do_s4(0)
            do_s4(1)
            do_attn(1, (0, 1))   # hides RS1_3
            do_oproj(2)
            do_mlp1(0, range(NM))      # hides RS1_2
            do_s4(3)
            do_s4(2)
            do_mlp1(1, range(NM))

            for cm in (ps_m1_cm, w1p_cm, attn_pool_cm,
                       ps_vt_cm, ps_o_cm, ps_st_cm, s4_t_cm,
                       s3_r1_cm, s2_misc_cm, s2_exp_cm, s2_vaug_cm):
                cm.__exit__(None, None, None)

            # ---- stage 6: MLP2 (m-major, all 8 PSUM banks accumulate) ----
            s6_w2_cm = tc.tile_pool(name="s6_w2", bufs=8)
            s6_w2 = s6_w2_cm.__enter__()
            s6_o_cm = tc.tile_pool(name="s6_o", bufs=2)
            s6_o = s6_o_cm.__enter__()
            ps_m2_cm = tc.tile_pool(name="ps_m2", bufs=1, space="PSUM")
            ps_m2 = ps_m2_cm.__enter__()

            pm2 = [ps_m2.tile([128, 1024], F32, tag=f"pm2_{tc_}",
                              name=f"pm2_{tc_}") for tc_ in range(4)]
            for m in range(NM):
                w2c = s6_w2.tile([128, D], BF, tag="w2c", name="w2c")
                nc.sync.dma_start(w2c[:],
                                   w2_in.ap()[m * 128:(m + 1) * 128, :])
                for tc_ in range(4):
                    for n2 in range(2):
                        nc.tensor.matmul(
                            pm2[tc_][:, n2 * 512:(n2 + 1) * 512],
                            g1_sb[:, m, tc_ * 128:(tc_ + 1) * 128],
                            w2c[:, n2 * 512:(n2 + 1) * 512],
                            start=(m == 0), stop=(m == NM - 1))
            for tc_ in range(4):
                ot = s6_o.tile([128, D], F32, tag="ot", name="ot")
                nc.vector.tensor_add(ot[:], x2_sb[:, tc_, :], pm2[tc_][:])
                nc.vector.tensor_add(ot[:], ot[:], b2b_sb[:])
                nc.sync.dma_start(
                    out_ext.ap()[tc_ * 128:(tc_ + 1) * 128, :], ot[:])

            for cm in (ps_m2_cm, s6_o_cm, s6_w2_cm, resid_pool_cm):
                cm.__exit__(None, None, None)

    nc.compile()
    _CACHE["nc"] = nc
    return nc


def shard_rows(c):
    """Global token rows owned by core c (four strided pieces of 128)."""
    return np.concatenate(
        [np.arange(q * 1024 + c * 128, q * 1024 + (c + 1) * 128)
         for q in range(4)])


def make_in_maps(x, ln1_g, ln1_b, w_qkv, w_out, ln2_g, ln2_b, w1, b1, w2, b2):
    import ml_dtypes
    bf16 = ml_dtypes.bfloat16
    fp8 = ml_dtypes.float8_e4m3
    x = np.asarray(x, np.float32)
    xf = np.ascontiguousarray(x.reshape(T, D))
    xt = np.ascontiguousarray(xf.T.astype(fp8))
    w_qkv_eff = np.asarray(w_qkv) * np.asarray(ln1_g)[:, None]
    bias_qkv = np.asarray(ln1_b) @ np.asarray(w_qkv)
    w1_eff = np.asarray(w1) * np.asarray(ln2_g)[:, None]
    bias_h1 = np.asarray(ln2_b) @ np.asarray(w1) + np.asarray(b1)
    w1b = np.ascontiguousarray(w1_eff.astype(bf16))
    b1gb = np.ascontiguousarray(bias_h1, np.float32).reshape(-1, 1)
    w2b = np.ascontiguousarray(np.asarray(w2).astype(bf16))
    b2b = np.tile(np.asarray(b2).astype(bf16)[None, :], (128, 1))
    km = np.arange(128)[:, None]
    qm = np.arange(512)[None, :]
    masks = np.stack([np.where(km + 128 * m <= qm, 0.0, -448.0).astype(fp8)
                      for m in range(4)])
    in_maps = []
    for c in range(NCORES):
        cs = slice(c * DLOC, (c + 1) * DLOC)
        wq = np.concatenate(
            [w_qkv_eff[:, cs], w_qkv_eff[:, D:][:, cs],
             w_qkv_eff[:, 2 * D:][:, cs]], axis=1)
        wq8 = (wq * WSC).astype(fp8)          # [D, 384] scaled fp8
        # SBUF layout [r, p, m, i, c] = wq8[p*256 + i*128 + r, m*128 + c]
        wq8_t = np.ascontiguousarray(
            wq8.reshape(ND // 2, 2, 128, 3, 128).transpose(2, 0, 3, 1, 4))
        bq = np.concatenate(
            [bias_qkv[cs], bias_qkv[D:][cs], bias_qkv[2 * D:][cs]])
        rows = shard_rows(c)
        in_maps.append({
            "xt": xt,
            "xsh": np.ascontiguousarray(xf[rows]),
            "xshb": np.ascontiguousarray(xf[rows].astype(bf16)),
            "wqkv": wq8_t,
            "nws": np.ascontiguousarray(
                (-(wq8.astype(np.float32) / WSC).sum(axis=0)).astype(
                    np.float32)).reshape(-1, 1),
            "bqkv": np.ascontiguousarray(bq, np.float32).reshape(-1, 1),
            "wout": np.ascontiguousarray(
                np.asarray(w_out)[cs].astype(bf16)),
            "w1": w1b, "b1g": b1gb, "w2": w2b,
            "b2b": b2b,
            "masks": masks,
        })
    return in_maps


def kernel(**inputs):
    nc = build()
    in_maps = make_in_maps(**inputs)
    res = bass_utils.run_bass_kernel_spmd(
        nc, in_maps, core_ids=list(range(NCORES)))
    out = np.empty((T, D), np.float32)
    for c in range(NCORES):
        out[shard_rows(c)] = res.results[c]["out"]
    return out.reshape(B, L, D).astype(np.float32)



# revision 67
# speedup vs baseline: 1.0944x; 1.0481x over previous
"""Trainium2 8-core kernel for a dense pre-norm transformer block.

Reference: h=LN1(x); qkv=h@w_qkv; causal MHA (16 heads, Dh=64);
x+=o@w_out; h2=LN2(x); x+=gelu(h2@w1+b1)@w2+b2.

Sharding (Megatron TP attention + data-parallel MLP):
  - heads 2c,2c+1 on core c (w_qkv column-shard, w_out row-shard);
    the attention out-projection partials are combined with four
    quarter-sized ReduceScatters (RS1), each fired as soon as its
    token quarter's out-projection finishes and hidden under the
    remaining attention / MLP1 compute.
  - residual stream token-sharded: core c owns the four strided
    pieces {q*1024 + c*128 .. +128}, q=0..3.
  - the MLP runs DATA-parallel: every core holds the full w1/w2 and
    computes the whole MLP for its own 512 tokens, so there is no
    AllGather and no second ReduceScatter; w1 (8MB) is DMA'd during
    attention into the SBUF freed by stage-1 pools, w2 streams
    through an 8-chunk ring during MLP2.  MLP2 accumulates m-major
    into all 8 PSUM banks (one [128,1024] fp32 tile per token piece).
  - LN1 stats (mean/rstd) are computed shard-locally with bn_stats
    (from a bf16 copy of x) and exchanged via a tiny AllGather at
    kernel start (preceded by a no-dep dummy AllGather that absorbs
    first-collective init/skew latency); LN gains/biases are folded
    into w_qkv/w1 host-side; LN1 mean-subtraction is folded into the
    qkv matmul as a rank-1 (-colsum(w) x mean) accumulation and the
    rstd scaling is applied to the matmul output.

Compute dtypes: the qkv matmul and the attention*V matmul run in fp8
(e4m3) with MatmulPerfMode.DoubleRow (256-deep contraction per pass,
2x PE throughput); weights are prescaled by 32 to dodge e4m3
subnormals and the scale is folded back via the rstd broadcast.
Scores stay bf16 (K=64, no DoubleRow win); the out-projection and
both MLP matmuls stay bf16 for accuracy.  PSUM accumulates fp32 and
the residual stream is fp32.

Attention scores are computed transposed ST=[k_pos, q_pos]; the two
heads run concurrently in the PE array via tile_position row-packing;
softmax denominator comes from a ones-column appended to V (vaug
padded to 80 cols so dual-fp8 LDWEIGHTS step%16==0 holds); causality
via additive -448 masks applied to the diagonal PSUM band before the
exp, which writes fp8 directly.
"""
import numpy as np

import concourse.bass as bass
import concourse.mybir as mybir
import concourse.tile as tile
from concourse import bacc
from concourse import bass_utils
from concourse.masks import make_identity

F32 = mybir.dt.float32
BF = mybir.dt.bfloat16
F8 = mybir.dt.float8e4
AF = mybir.ActivationFunctionType
DR = mybir.MatmulPerfMode.DoubleRow
WSC = 32.0  # fp8 weight prescale (avoids e4m3 subnormals for 0.02-scale w)

NCORES = 8
B, L, D = 2, 2048, 1024
T = B * L              # 4096 tokens
TSH = T // NCORES      # 512 tokens per core (4 pieces of 128)
DH = 64                # head dim
HL = 2                 # heads per core
DLOC = HL * DH         # 128 local head features
MLPH = 4096 // NCORES  # 512 local hidden
LN_EPS = 1e-5
NT = T // 512          # 8 token tiles of 512
ND = D // 128          # 8 feature chunks
QT = L // 512          # 4 q-tiles per batch

_CACHE = {}


def build():
    if "nc" in _CACHE:
        return _CACHE["nc"]
    nc = bacc.Bacc("TRN2", target_bir_lowering=False, debug=False,
                   num_devices=NCORES)

    xt_in = nc.dram_tensor("xt", [D, T], F8, kind="ExternalInput")
    xsh_in = nc.dram_tensor("xsh", [TSH, D], F32, kind="ExternalInput")
    xshb_in = nc.dram_tensor("xshb", [TSH, D], BF, kind="ExternalInput")
    wqkv_in = nc.dram_tensor("wqkv", [128, ND // 2, 3, 2, 128], F8,
                             kind="ExternalInput")
    nws_in = nc.dram_tensor("nws", [3 * DLOC, 1], F32, kind="ExternalInput")
    bqkv_in = nc.dram_tensor("bqkv", [3 * DLOC, 1], F32, kind="ExternalInput")
    wout_in = nc.dram_tensor("wout", [DLOC, D], BF, kind="ExternalInput")
    w1_in = nc.dram_tensor("w1", [D, 4 * D], BF, kind="ExternalInput")
    b1g_in = nc.dram_tensor("b1g", [4 * D, 1], F32, kind="ExternalInput")
    w2_in = nc.dram_tensor("w2", [4 * D, D], BF, kind="ExternalInput")
    b2b_in = nc.dram_tensor("b2b", [128, D], BF, kind="ExternalInput")
    masks_in = nc.dram_tensor("masks", [4, 128, 512], F8, kind="ExternalInput")
    out_ext = nc.dram_tensor("out", [TSH, D], F32, kind="ExternalOutput")

    rg = [list(range(NCORES))]

    with tile.TileContext(nc) as tc:
        with (
            tc.tile_pool(name="const", bufs=1) as const,
            tc.tile_pool(name="wpool", bufs=1) as wpool,
            tc.tile_pool(name="dram", bufs=1, space="DRAM") as dram,
        ):
            # ---- DRAM scratch for collectives ----
            st_ag_in = dram.tile([8, 128], BF)       # (piece q, mean/rstd)
            st_ag_out = dram.tile([64, 128], BF, addr_space="Shared")
            rs1_in = [dram.tile([1024, D], BF, name=f"rs1_in{q}")
                      for q in range(4)]
            rs1_out = [dram.tile([128, D], BF, name=f"rs1_out{q}")
                       for q in range(4)]

            warm_in = dram.tile([8, 16], BF)
            warm_out = dram.tile([64, 16], BF, addr_space="Shared")

            # ---- constants ----
            ident = const.tile([128, 128], F32)
            make_identity(nc, ident[:])
            ident_bf = const.tile([128, 128], BF)
            make_identity(nc, ident_bf[:])
            ones_row = const.tile([1, 128], BF)
            nc.vector.memset(ones_row[:], 1.0)
            sc_row = const.tile([1, 128], BF)
            nc.vector.memset(sc_row[:], 1.0 / WSC)
            eps128 = const.tile([128, 1], F32)
            nc.vector.memset(eps128[:], LN_EPS)
            masks_sb = [const.tile([128, 512], F8, name=f"mask{m}")
                        for m in range(4)]

            def load_masks():
                for m in range(4):
                    nc.sync.dma_start(masks_sb[m][:], masks_in.ap()[m])

            # ---- weights resident in SBUF ----
            # wqkv_sb[r, p, m, i, c] = WSC * w_eff[p*256 + i*128 + r,
            #                                      m*128 + c], fp8
            # (pair k-tiles contiguous for dual-fp8 ldweights)
            wqkv_sb = wpool.tile([128, ND // 2, 3, 2, 128], F8, name="wqkv8")
            nc.sync.dma_start(wqkv_sb[:], wqkv_in.ap())
            nws_sb = []
            for m in range(3):
                nt_ = wpool.tile([128, 1], F32, name=f"nws{m}")
                nc.sync.dma_start(nt_[:],
                                  nws_in.ap()[m * 128:(m + 1) * 128, :])
                nws_sb.append(nt_)
            bqkv_sb = []
            for m in range(3):
                bt = wpool.tile([128, 1], F32, name=f"bqkv{m}")
                nc.sync.dma_start(bt[:], bqkv_in.ap()[m * 128:(m + 1) * 128, :])
                bqkv_sb.append(bt)
            NM = 4 * D // 128       # 32 hidden chunks (full MLP per core)
            wout_sb = wpool.tile([DLOC, D], BF)
            b2b_sb = wpool.tile([128, D], BF, name="b2b")

            def load_late_weights():
                nc.sync.dma_start(wout_sb[:], wout_in.ap())
                nc.sync.dma_start(b2b_sb[:], b2b_in.ap())
                for q in range(4):
                    nc.sync.dma_start(
                        xsv[:, q, :], xsh_in.ap()[q * 128:(q + 1) * 128, :])

            resid_pool_cm = tc.tile_pool(name="resid", bufs=1)
            resid_pool = resid_pool_cm.__enter__()
            xsv = resid_pool.tile([128, 4, D], F32)   # my shard of x
            x2_sb = resid_pool.tile([128, 4, D], F32)
            h2T = resid_pool.tile([128, ND, 512], BF)     # LN2(x2)^T shard
            g1_sb = resid_pool.tile([128, NM, 512], BF)   # gelu acts [h, tok]

            # ========== stage 0: shard-local LN1 stats + tiny AG ==========
            s0x_cm = tc.tile_pool(name="s0x", bufs=1)
            s0x = s0x_cm.__enter__()
            xsb = s0x.tile([128, 4, D], BF)    # bf16 x copy for LN1 stats
            s0_cm = tc.tile_pool(name="s0", bufs=2)
            s0 = s0_cm.__enter__()
            ps0_cm = tc.tile_pool(name="ps0", bufs=2, space="PSUM")
            ps0 = ps0_cm.__enter__()
            # absorb first-collective init latency with a no-dep dummy
            wtile = s0.tile([8, 16], BF, tag="wtile", name="wtile")
            nc.vector.memset(wtile[:], 0.0)
            nc.scalar.dma_start(warm_in[:], wtile[:])
            nc.gpsimd.collective_compute(
                "AllGather", mybir.AluOpType.bypass, replica_groups=rg,
                ins=[warm_in[:].opt()], outs=[warm_out[:].opt()])
            for q in range(4):
                nc.gpsimd.dma_start(
                    xsb[:, q, :], xshb_in.ap()[q * 128:(q + 1) * 128, :])
                stats = s0.tile([128, 2, 6], F32, tag="stats", name="stats")
                xv = xsb[:, q, :].rearrange("p (s f) -> p s f", s=2)
                for s in range(2):
                    nc.vector.bn_stats(stats[:, s, :], xv[:, s, :])
                mv = s0.tile([128, 2], F32, tag="mv", name="mv")
                nc.vector.bn_aggr(mv[:], stats[:])
                rstd0 = s0.tile([128, 1], F32, tag="rstd0", name="rstd0")
                nc.scalar.activation(rstd0[:], mv[:, 1:2], AF.Sqrt,
                                     bias=eps128[:])
                nc.vector.reciprocal_approx_fast(rstd0[:], rstd0[:])
                st2 = s0.tile([128, 2], BF, tag="st2", name="st2")
                nc.vector.tensor_copy(st2[:, 0:1], mv[:, 0:1])
                nc.vector.tensor_copy(st2[:, 1:2], rstd0[:])
                stp = ps0.tile([2, 128], BF, tag="stp", name="stp")
                nc.tensor.transpose(stp[:], st2[:], ident_bf[:])
                sts = s0.tile([2, 128], BF, tag="sts", name="sts")
                nc.vector.tensor_copy(sts[:], stp[:])
                nc.scalar.dma_start(st_ag_in[2 * q:2 * q + 2, :], sts[:])
            nc.gpsimd.collective_compute(
                "AllGather", mybir.AluOpType.bypass, replica_groups=rg,
                ins=[st_ag_in[:].opt()], outs=[st_ag_out[:].opt()])
            ps0_cm.__exit__(None, None, None)
            s0_cm.__exit__(None, None, None)
            s0x_cm.__exit__(None, None, None)

            # st_ag_out rows: c*8 + q*2 + {0:mean, 1:rstd}
            st_view = st_ag_out[:].rearrange("(c x) f -> c x f", x=8)

            praws = {}

            def do_s1_mm(tt):
                q4, h4 = tt // 2, tt % 2
                xts = s1_x.tile([128, ND, 512], F8, tag="xts")
                eng = (nc.sync, nc.scalar)[tt % 2]
                eng.dma_start(
                    xts[:],
                    xt_in.ap()[:, tt * 512:(tt + 1) * 512].rearrange(
                        "(c p) t -> p c t", p=128))
                praws[tt] = (None, None, [])
                for m in range(3):
                    ps_q = ps_qkv.tile([128, 512], F32, tag="ps_q",
                                       name="ps_q")
                    for p in range(ND // 2):
                        nc.tensor.matmul(
                            ps_q[:],
                            wqkv_sb[:, p, m, :, :],
                            xts[:, 2 * p:2 * p + 2, :],
                            start=(p == 0), stop=(p == ND // 2 - 1),
                            perf_mode=DR)
                    praw = praw_pool.tile([128, 512], BF, tag="praw",
                                          name="praw")
                    nc.scalar.copy(praw[:], ps_q[:])
                    praws[tt][2].append(praw)

            def do_s1_fin(tt):
                q4, h4 = tt // 2, tt % 2
                _, _, praw3 = praws[tt]
                mean_bf = s1_stat.tile([1, 4, 128], BF, tag=f"mean_bf{tt}",
                                       name=f"mean_bf{tt}")
                rstd_bf = s1_stat.tile([1, 4, 128], BF, tag=f"rstd_bf{tt}",
                                       name=f"rstd_bf{tt}")
                nc.gpsimd.dma_start(
                    mean_bf[:], st_view[4 * h4:4 * h4 + 4, 2 * q4, :])
                nc.gpsimd.dma_start(
                    rstd_bf[:], st_view[4 * h4:4 * h4 + 4, 2 * q4 + 1, :])
                mean_v = mean_bf[:].rearrange("p a f -> p (a f)")
                rstd_v = rstd_bf[:].rearrange("p a f -> p (a f)")
                mr = s1_stat.tile([1, 512], BF, tag="mr", name="mr")
                nc.vector.tensor_mul(mr[:], mean_v, rstd_v)
                rstd_b = ps_st.tile([128, 512], F32, tag="pst",
                                    name="rstd_b")
                nc.tensor.matmul(rstd_b[:], sc_row[:], rstd_v,
                                 start=True, stop=True)
                rstd_bc = s1_tmp.tile([128, 512], BF, tag="rstd_bc")
                if tt >= 4:
                    nc.scalar.copy(rstd_bc[:], rstd_b[:])
                else:
                    nc.vector.tensor_copy(rstd_bc[:], rstd_b[:])
                mr_b = ps_st.tile([128, 512], F32, tag="pst", name="mr_b")
                nc.tensor.matmul(mr_b[:], ones_row[:], mr[:],
                                 start=True, stop=True)
                for m in range(3):
                    u = s1_tmp.tile([128, 512], BF, tag="pre", name="u")
                    nc.vector.tensor_mul(u[:], praw3[m][:], rstd_bc[:])
                    pre = s1_tmp.tile([128, 512], BF, tag="pre2",
                                      name="pre2")
                    nc.vector.scalar_tensor_tensor(
                        out=pre[:], in0=mr_b[:], scalar=nws_sb[m][:],
                        in1=u[:], op0=mybir.AluOpType.mult,
                        op1=mybir.AluOpType.add)
                    nc.vector.tensor_scalar(
                        out=qkvT[m][:, tt * 512:(tt + 1) * 512], in0=pre[:],
                        scalar1=bqkv_sb[m][:], scalar2=None,
                        op0=mybir.AluOpType.add)
                del praws[tt]
            # ============ stage 2/3/4 pools ============
            s2_vaug_cm = tc.tile_pool(name="s2_vaug", bufs=1)
            s2_vaug = s2_vaug_cm.__enter__()
            s2_exp_cm = tc.tile_pool(name="s2_exp", bufs=2)
            s2_exp = s2_exp_cm.__enter__()
            s2_misc_cm = tc.tile_pool(name="s2_misc", bufs=1)
            s2_misc = s2_misc_cm.__enter__()
            s3_r1_cm = tc.tile_pool(name="s3_r1", bufs=2)
            s3_r1 = s3_r1_cm.__enter__()
            s4_t_cm = tc.tile_pool(name="s4_t", bufs=1)
            s4_t = s4_t_cm.__enter__()
            ps_st_cm = tc.tile_pool(name="ps_st", bufs=3, space="PSUM")
            ps_st = ps_st_cm.__enter__()
            ps_o_cm = tc.tile_pool(name="ps_o", bufs=1, space="PSUM")
            ps_o = ps_o_cm.__enter__()
            ps_vt_cm = tc.tile_pool(name="ps_vt", bufs=1, space="PSUM")
            ps_vt = ps_vt_cm.__enter__()

            # persistent activations
            attn_pool_cm = tc.tile_pool(name="attn", bufs=1)
            attn_pool = attn_pool_cm.__enter__()
            qkvT = []
            for m in range(3):
                t_ = attn_pool.tile([128, T], BF, name=f"qkvT{m}")
                qkvT.append(t_)
            oT = attn_pool.tile([128, T], BF)

            # ================= stage 1 pools (popped mid-kernel) ==========
            s1_x_cm = tc.tile_pool(name="s1_x", bufs=2)
            s1_x = s1_x_cm.__enter__()
            s1_tmp_cm = tc.tile_pool(name="s1_tmp", bufs=3)
            s1_tmp = s1_tmp_cm.__enter__()
            s1_stat_cm = tc.tile_pool(name="s1_stat", bufs=1)
            s1_stat = s1_stat_cm.__enter__()
            ps_qkv_cm = tc.tile_pool(name="ps_qkv", bufs=2, space="PSUM")
            ps_qkv = ps_qkv_cm.__enter__()
            praw_pool_cm = tc.tile_pool(name="s1_praw", bufs=24)
            praw_pool = praw_pool_cm.__enter__()

            vaugs = {}

            def do_vaug(b, tl):
                """V-transposes for 512-token tile tl (4 k-chunks) of batch b."""
                tok0 = b * L
                if b not in vaugs:
                    vaug = s2_vaug.tile([128, HL, L // 256, 2, DH + 16], F8,
                                        tag=f"vaug{b}", name=f"vaug{b}")
                    nc.vector.memset(vaug[:, :, :, :, DH:DH + 1], 1.0)
                    nc.vector.memset(vaug[:, :, :, :, DH + 1:DH + 16], 0.0)
                    vaugs[b] = vaug
                vaug = vaugs[b]
                for hl in range(HL):
                    hrow = hl * DH
                    vT_u = qkvT[2][hrow:hrow + DH, tok0:tok0 + L]
                    for kc in range(4 * tl, 4 * tl + 4):
                        pv = ps_vt.tile([128, DH], BF, tag="pv",
                                        name="pv")
                        nc.tensor.transpose(
                            pv[:], vT_u[:, kc * 128:(kc + 1) * 128],
                            ident_bf[hrow:hrow + DH, hrow:hrow + DH])
                        if b == 0:
                            nc.scalar.copy(
                                vaug[:, hl, kc // 2, kc % 2, 0:DH], pv[:])
                        else:
                            nc.vector.tensor_copy(
                                vaug[:, hl, kc // 2, kc % 2, 0:DH], pv[:])

            def do_attn(b, js, fill=None):
                tok0 = b * L
                vaug = vaugs[b]
                for j in js:
                    nk = 4 * (j + 1)
                    po = [ps_o.tile([DH + 16, 512], F32, tag=f"po{hl}",
                                    name=f"po{hl}") for hl in range(HL)]
                    for kcp in range(nk // 2):
                        kc0 = 2 * kcp
                        dm0 = kc0 - (nk - 4)
                        col0 = 128 * dm0 if dm0 > 0 else 0
                        w = 512 - col0
                        ests = []
                        for hl in range(HL):
                            hrow = hl * DH
                            qsl = qkvT[0][hrow:hrow + DH,
                                          tok0 + j * 512 + col0:
                                          tok0 + (j + 1) * 512]
                            est = s2_exp.tile([128, 2, 512], F8,
                                              tag=f"est{hl}", name=f"est{hl}")
                            for i in range(2):
                                kc = kc0 + i
                                dm = kc - (nk - 4)
                                ksl = qkvT[1][hrow:hrow + DH,
                                              tok0 + kc * 128:
                                              tok0 + (kc + 1) * 128]
                                pst = ps_st.tile([128, 512], F32, tag="pst",
                                                 name="pst")
                                nc.tensor.matmul(pst[:, :w], ksl, qsl,
                                                 start=True, stop=True,
                                                 tile_position=(hrow, 0))
                                if dm >= 0:
                                    mw = 128 * (dm + 1) - col0
                                    nc.vector.tensor_add(
                                        pst[:, :mw], pst[:, :mw],
                                        masks_sb[dm][:, col0:col0 + mw])
                                nc.scalar.activation(est[:, i, :w],
                                                     pst[:, :w],
                                                     AF.Exp, scale=0.125)
                            ests.append(est)
                        for hl in range(HL):
                            nc.tensor.matmul(po[hl][:, col0:],
                                             vaug[:, hl, kcp, :, :],
                                             ests[hl][:, :, :w],
                                             start=(kcp == 0),
                                             stop=(kcp == nk // 2 - 1),
                                             perf_mode=DR)
                        if fill is not None:
                            fill()
                    for hl in range(HL):
                        hrow = hl * DH
                        den = s2_misc.tile([1, 512], F32, tag="den",
                                           name="den")
                        nc.vector.tensor_copy(den[:], po[hl][DH:DH + 1, :])
                        rec1 = s2_misc.tile([1, 512], F32, tag="rec1",
                                            name="rec1")
                        nc.vector.reciprocal_approx_fast(rec1[:], den[:])
                        rec1b = s2_misc.tile([1, 512], BF, tag="rec1b",
                                             name="rec1b")
                        nc.vector.tensor_copy(rec1b[:], rec1[:])
                        rec_b = ps_vt.tile([64, 512], F32, tag="pv",
                                           name="rec_b")
                        nc.tensor.matmul(rec_b[:], ones_row[0:1, 0:64],
                                         rec1b[:], start=True, stop=True)
                        rec_sb = s2_misc.tile([64, 512], BF, tag="rec_sb",
                                              name="rec_sb")
                        nc.vector.tensor_copy(rec_sb[:], rec_b[:])
                        nc.vector.tensor_mul(
                            oT[hrow:hrow + DH,
                               tok0 + j * 512:tok0 + (j + 1) * 512],
                            po[hl][0:DH, :], rec_sb[:])

            def do_oproj(q):
                """out-projection for quarter q (tokens q*1024..+1024) + RS."""
                for tch in range(8):
                    row0 = q * 1024 + tch * 128
                    r1 = s3_r1.tile([128, D], BF, tag="r1", name="r1")
                    for n in range(2):
                        pop = ps_st.tile([128, 512], F32, tag="pst",
                                         name="pop")
                        nc.tensor.matmul(pop[:], oT[:, row0:row0 + 128],
                                         wout_sb[:, n * 512:(n + 1) * 512],
                                         start=True, stop=True)
                        if n == 0:
                            nc.vector.tensor_copy(
                                r1[:, n * 512:(n + 1) * 512], pop[:])
                        else:
                            nc.scalar.copy(
                                r1[:, n * 512:(n + 1) * 512], pop[:])
                    nc.gpsimd.dma_start(
                        rs1_in[q][tch * 128:(tch + 1) * 128, :], r1[:])
                nc.gpsimd.collective_compute(
                    "ReduceScatter", mybir.AluOpType.add, replica_groups=rg,
                    ins=[rs1_in[q][:].opt()], outs=[rs1_out[q][:].opt()])

            def do_s4(q):
                """residual + LN2 + transpose for my piece of quarter q."""
                r1s = s4_t.tile([128, D], BF, tag="r1s", name="r1s")
                nc.sync.dma_start(r1s[:], rs1_out[q][:])
                nc.vector.tensor_add(x2_sb[:, q, :], xsv[:, q, :], r1s[:])
                stats = s4_t.tile([128, 2, 6], F32, tag="stats", name="stats")
                x2v = x2_sb[:, q, :].rearrange("p (s f) -> p s f", s=2)
                for s in range(2):
                    nc.vector.bn_stats(stats[:, s, :], x2v[:, s, :])
                mv = s4_t.tile([128, 2], F32, tag="mv", name="mv")
                nc.vector.bn_aggr(mv[:], stats[:])
                rstd2 = s4_t.tile([128, 1], F32, tag="rstd2", name="rstd2")
                nc.scalar.activation(rstd2[:], mv[:, 1:2], AF.Sqrt,
                                     bias=eps128[:])
                nc.vector.reciprocal_approx_fast(rstd2[:], rstd2[:])
                h2 = s4_t.tile([128, D], F32, tag="h2", name="h2")
                nc.vector.tensor_scalar(
                    out=h2[:], in0=x2_sb[:, q, :], scalar1=mv[:, 0:1],
                    scalar2=rstd2[:], op0=mybir.AluOpType.subtract,
                    op1=mybir.AluOpType.mult)
                for d in range(ND):
                    pt = ps_vt.tile([128, 128], F32, tag="pv", name="pt")
                    nc.tensor.transpose(
                        pt[:], h2[:, d * 128:(d + 1) * 128], ident[:])
                    nc.vector.tensor_copy(h2T[:, d, q * 128:(q + 1) * 128],
                                          pt[:])

            # ---------------- pipelined schedule (front) ----------------
            for tt in range(NT):
                do_s1_mm(tt)
            load_masks()
            load_late_weights()
            for tt in range(4):
                do_s1_fin(tt)
                do_vaug(0, tt)
            do_attn(0, (0, 1))
            do_oproj(0)
            do_attn(0, (2, 3))
            for tt in range(4, NT):
                do_s1_fin(tt)
                do_vaug(1, tt - 4)

            # s1 done: free its SBUF/PSUM, bring in the full w1 for DP-MLP
            praw_pool_cm.__exit__(None, None, None)
            for cm in (ps_qkv_cm, s1_stat_cm, s1_tmp_cm, s1_x_cm):
                cm.__exit__(None, None, None)
            w1p_cm = tc.tile_pool(name="w1p", bufs=1)
            w1p = w1p_cm.__enter__()
            w1_sb = [w1p.tile([128, 4 * D], BF, name=f"w1_{d}")
                     for d in range(ND)]
            b1g_sb = w1p.tile([128, NM], F32, name="b1g")
            nc.sync.dma_start(
                b1g_sb[:],
                b1g_in.ap().rearrange("(m r) o -> r (m o)", r=128))
            for d in range(ND):
                nc.gpsimd.dma_start(w1_sb[d][:],
                                    w1_in.ap()[d * 128:(d + 1) * 128, :])
            ps_m1_cm = tc.tile_pool(name="ps_m1", bufs=2, space="PSUM")
            ps_m1 = ps_m1_cm.__enter__()

            def do_mlp1(qp, ms, raw=False):
                """MLP1 for token half qp (256 cols), hidden chunks ms.
                raw=True defers GELU: PSUM is copied to g1 by vector and a
                later batched gelu pass applies the activation in place
                (avoids exp<->gelu ACT-table thrash mid-attention)."""
                c0 = qp * 256
                for m in ms:
                    pm1 = ps_m1.tile([128, 256], F32, tag="pm1", name="pm1")
                    for d in range(ND):
                        nc.tensor.matmul(
                            pm1[:], w1_sb[d][:, m * 128:(m + 1) * 128],
                            h2T[:, d, c0:c0 + 256], start=(d == 0),
                            stop=(d == ND - 1))
                    if raw:
                        nc.vector.tensor_copy(g1_sb[:, m, c0:c0 + 256],
                                              pm1[:])
                    else:
                        nc.scalar.activation(g1_sb[:, m, c0:c0 + 256],
                                             pm1[:], AF.Gelu,
                                             bias=b1g_sb[:, m:m + 1])

            def do_gelu_pass(qp, ms):
                c0 = qp * 256
                for m in ms:
                    nc.scalar.activation(g1_sb[:, m, c0:c0 + 256],
                                         g1_sb[:, m, c0:c0 + 256],
                                         AF.Gelu, bias=b1g_sb[:, m:m + 1])

            do_oproj(1)
            do_attn(1, (2, 3))   # hides RS1_0 + RS1_1
            do_oproj(3)
            do_s4(0)
            do_s4(1)
            do_attn(1, (0, 1))   # hides RS1_3
            do_oproj(2)
            do_mlp1(0, range(0, 8))    # hides RS1_2
            do_s4(3)
            do_mlp1(0, range(8, 16))
            do_s4(2)
            do_mlp1(0, range(16, NM))
            do_mlp1(1, range(NM))

            for cm in (ps_m1_cm, w1p_cm, attn_pool_cm,
                       ps_vt_cm, ps_o_cm, ps_st_cm, s4_t_cm,
                       s3_r1_cm, s2_misc_cm, s2_exp_cm, s2_vaug_cm):
                cm.__exit__(None, None, None)

            # ---- stage 6: MLP2 (m-major, all 8 PSUM banks accumulate) ----
            s6_w2_cm = tc.tile_pool(name="s6_w2", bufs=8)
            s6_w2 = s6_w2_cm.__enter__()
            s6_o_cm = tc.tile_pool(name="s6_o", bufs=2)
            s6_o = s6_o_cm.__enter__()
            ps_m2_cm = tc.tile_pool(name="ps_m2", bufs=1, space="PSUM")
            ps_m2 = ps_m2_cm.__enter__()

            pm2 = [ps_m2.tile([128, 1024], F32, tag=f"pm2_{tc_}",
                              name=f"pm2_{tc_}") for tc_ in range(4)]
            for m in range(NM):
                w2c = s6_w2.tile([128, D], BF, tag="w2c", name="w2c")
                nc.sync.dma_start(w2c[:],
                                   w2_in.ap()[m * 128:(m + 1) * 128, :])
                for tc_ in range(4):
                    for n2 in range(2):
                        nc.tensor.matmul(
                            pm2[tc_][:, n2 * 512:(n2 + 1) * 512],
                            g1_sb[:, m, tc_ * 128:(tc_ + 1) * 128],
                            w2c[:, n2 * 512:(n2 + 1) * 512],
                            start=(m == 0), stop=(m == NM - 1))
            for tc_ in range(4):
                ot = s6_o.tile([128, D], F32, tag="ot", name="ot")
                nc.vector.tensor_add(ot[:], x2_sb[:, tc_, :], pm2[tc_][:])
                nc.vector.tensor_add(ot[:], ot[:], b2b_sb[:])
                nc.sync.dma_start(
                    out_ext.ap()[tc_ * 128:(tc_ + 1) * 128, :], ot[:])

            for cm in (ps_m2_cm, s6_o_cm, s6_w2_cm, resid_pool_cm):
                cm.__exit__(None, None, None)

    nc.compile()
    _CACHE["nc"] = nc
    return nc


def shard_rows(c):
    """Global token rows owned by core c (four strided pieces of 128)."""
    return np.concatenate(
        [np.arange(q * 1024 + c * 128, q * 1024 + (c + 1) * 128)
         for q in range(4)])


def make_in_maps(x, ln1_g, ln1_b, w_qkv, w_out, ln2_g, ln2_b, w1, b1, w2, b2):
    import ml_dtypes
    bf16 = ml_dtypes.bfloat16
    fp8 = ml_dtypes.float8_e4m3
    x = np.asarray(x, np.float32)
    xf = np.ascontiguousarray(x.reshape(T, D))
    xt = np.ascontiguousarray(xf.T.astype(fp8))
    w_qkv_eff = np.asarray(w_qkv) * np.asarray(ln1_g)[:, None]
    bias_qkv = np.asarray(ln1_b) @ np.asarray(w_qkv)
    w1_eff = np.asarray(w1) * np.asarray(ln2_g)[:, None]
    bias_h1 = np.asarray(ln2_b) @ np.asarray(w1) + np.asarray(b1)
    w1b = np.ascontiguousarray(w1_eff.astype(bf16))
    b1gb = np.ascontiguousarray(bias_h1, np.float32).reshape(-1, 1)
    w2b = np.ascontiguousarray(np.asarray(w2).astype(bf16))
    b2b = np.tile(np.asarray(b2).astype(bf16)[None, :], (128, 1))
    km = np.arange(128)[:, None]
    qm = np.arange(512)[None, :]
    masks = np.stack([np.where(km + 128 * m <= qm, 0.0, -448.0).astype(fp8)
                      for m in range(4)])
    in_maps = []
    for c in range(NCORES):
        cs = slice(c * DLOC, (c + 1) * DLOC)
        wq = np.concatenate(
            [w_qkv_eff[:, cs], w_qkv_eff[:, D:][:, cs],
             w_qkv_eff[:, 2 * D:][:, cs]], axis=1)
        wq8 = (wq * WSC).astype(fp8)          # [D, 384] scaled fp8
        # SBUF layout [r, p, m, i, c] = wq8[p*256 + i*128 + r, m*128 + c]
        wq8_t = np.ascontiguousarray(
            wq8.reshape(ND // 2, 2, 128, 3, 128).transpose(2, 0, 3, 1, 4))
        bq = np.concatenate(
            [bias_qkv[cs], bias_qkv[D:][cs], bias_qkv[2 * D:][cs]])
        rows = shard_rows(c)
        in_maps.append({
            "xt": xt,
            "xsh": np.ascontiguousarray(xf[rows]),
            "xshb": np.ascontiguousarray(xf[rows].astype(bf16)),
            "wqkv": wq8_t,
            "nws": np.ascontiguousarray(
                (-(wq8.astype(np.float32) / WSC).sum(axis=0)).astype(
                    np.float32)).reshape(-1, 1),
            "bqkv": np.ascontiguousarray(bq, np.float32).reshape(-1, 1),
            "wout": np.ascontiguousarray(
                np.asarray(w_out)[cs].astype(bf16)),
            "w1": w1b, "b1g": b1gb, "w2": w2b,
            "b2b": b2b,
            "masks": masks,
        })
    return in_maps


def kernel(**inputs):
    nc = build()
    in_maps = make_in_maps(**inputs)
    res = bass_utils.run_bass_kernel_spmd(
        nc, in_maps, core_ids=list(range(NCORES)))
    out = np.empty((T, D), np.float32)
    for c in range(NCORES):
        out[shard_rows(c)] = res.results[c]["out"]
    return out.reshape(B, L, D).astype(np.float32)



# revision 68
# speedup vs baseline: 1.1925x; 1.0896x over previous
"""Trainium2 8-core kernel for a dense pre-norm transformer block.

Reference: h=LN1(x); qkv=h@w_qkv; causal MHA (16 heads, Dh=64);
x+=o@w_out; h2=LN2(x); x+=gelu(h2@w1+b1)@w2+b2.

Sharding (Megatron TP attention + data-parallel MLP):
  - heads 2c,2c+1 on core c (w_qkv column-shard, w_out row-shard);
    the attention out-projection partials are combined with four
    quarter-sized ReduceScatters (RS1), each fired as soon as its
    token quarter's out-projection finishes and hidden under the
    remaining attention / MLP1 compute.
  - residual stream token-sharded: core c owns the four strided
    pieces {q*1024 + c*128 .. +128}, q=0..3.
  - the MLP runs DATA-parallel: every core holds the full w1/w2 and
    computes the whole MLP for its own 512 tokens, so there is no
    AllGather and no second ReduceScatter; w1 (8MB) is DMA'd during
    attention into the SBUF freed by stage-1 pools, w2 streams
    through an 8-chunk ring during MLP2.  MLP2 accumulates m-major
    into all 8 PSUM banks (one [128,1024] fp32 tile per token piece).
  - LN1 stats (mean/rstd) are computed shard-locally with bn_stats
    (from a bf16 copy of x) and exchanged via a tiny AllGather at
    kernel start (preceded by a no-dep dummy AllGather that absorbs
    first-collective init/skew latency); LN gains/biases are folded
    into w_qkv/w1 host-side; LN1 mean-subtraction is folded into the
    qkv matmul as a rank-1 (-colsum(w) x mean) accumulation and the
    rstd scaling is applied to the matmul output.

Compute dtypes: the qkv matmul and the attention*V matmul run in fp8
(e4m3) with MatmulPerfMode.DoubleRow (256-deep contraction per pass,
2x PE throughput); weights are prescaled by 32 to dodge e4m3
subnormals and the scale is folded back via the rstd broadcast.
Scores stay bf16 (K=64, no DoubleRow win); the out-projection and
both MLP matmuls stay bf16 for accuracy.  PSUM accumulates fp32 and
the residual stream is fp32.

Attention scores are computed transposed ST=[k_pos, q_pos]; the two
heads run concurrently in the PE array via tile_position row-packing;
softmax denominator comes from a ones-column appended to V (vaug
padded to 80 cols so dual-fp8 LDWEIGHTS step%16==0 holds); causality
via additive -448 masks applied to the diagonal PSUM band before the
exp, which writes fp8 directly.
"""
import numpy as np

import concourse.bass as bass
import concourse.mybir as mybir
import concourse.tile as tile
from concourse import bacc
from concourse import bass_utils
from concourse.masks import make_identity

F32 = mybir.dt.float32
BF = mybir.dt.bfloat16
F8 = mybir.dt.float8e4
AF = mybir.ActivationFunctionType
DR = mybir.MatmulPerfMode.DoubleRow
WSC = 32.0  # fp8 weight prescale (avoids e4m3 subnormals for 0.02-scale w)

NCORES = 8
B, L, D = 2, 2048, 1024
T = B * L              # 4096 tokens
TSH = T // NCORES      # 512 tokens per core (4 pieces of 128)
DH = 64                # head dim
HL = 2                 # heads per core
DLOC = HL * DH         # 128 local head features
MLPH = 4096 // NCORES  # 512 local hidden
LN_EPS = 1e-5
NT = T // 512          # 8 token tiles of 512
ND = D // 128          # 8 feature chunks
QT = L // 512          # 4 q-tiles per batch

_CACHE = {}


def build():
    if "nc" in _CACHE:
        return _CACHE["nc"]
    nc = bacc.Bacc("TRN2", target_bir_lowering=False, debug=False,
                   num_devices=NCORES)

    xt_in = nc.dram_tensor("xt", [D, T], F8, kind="ExternalInput")
    xsh_in = nc.dram_tensor("xsh", [TSH, D], F32, kind="ExternalInput")
    xshb_in = nc.dram_tensor("xshb", [TSH, D], BF, kind="ExternalInput")
    wqkv_in = nc.dram_tensor("wqkv", [128, ND // 2, 3, 2, 128], F8,
                             kind="ExternalInput")
    nws_in = nc.dram_tensor("nws", [3 * DLOC, 1], F32, kind="ExternalInput")
    bqkv_in = nc.dram_tensor("bqkv", [3 * DLOC, 1], F32, kind="ExternalInput")
    wout_in = nc.dram_tensor("wout", [DLOC, D], BF, kind="ExternalInput")
    w1_in = nc.dram_tensor("w1", [D, 4 * D], BF, kind="ExternalInput")
    b1g_in = nc.dram_tensor("b1g", [4 * D, 1], F32, kind="ExternalInput")
    w2_in = nc.dram_tensor("w2", [4 * D, D], BF, kind="ExternalInput")
    b2b_in = nc.dram_tensor("b2b", [128, D], BF, kind="ExternalInput")
    masks_in = nc.dram_tensor("masks", [4, 128, 512], F8, kind="ExternalInput")
    out_ext = nc.dram_tensor("out", [TSH, D], F32, kind="ExternalOutput")

    rg = [list(range(NCORES))]

    with tile.TileContext(nc) as tc:
        with (
            tc.tile_pool(name="const", bufs=1) as const,
            tc.tile_pool(name="wpool", bufs=1) as wpool,
            tc.tile_pool(name="dram", bufs=1, space="DRAM") as dram,
        ):
            # ---- DRAM scratch for collectives ----
            st_ag_in = dram.tile([8, 128], BF)       # (piece q, mean/rstd)
            st_ag_out = dram.tile([64, 128], BF, addr_space="Shared")
            rs1_in = [dram.tile([1024, D], BF, name=f"rs1_in{q}")
                      for q in range(4)]
            rs1_out = [dram.tile([128, D], BF, name=f"rs1_out{q}")
                       for q in range(4)]

            warm_in = dram.tile([8, 16], BF)
            warm_out = dram.tile([64, 16], BF, addr_space="Shared")

            # ---- constants ----
            ident = const.tile([128, 128], F32)
            make_identity(nc, ident[:])
            ident_bf = const.tile([128, 128], BF)
            make_identity(nc, ident_bf[:])
            ones_row = const.tile([1, 128], BF)
            nc.vector.memset(ones_row[:], 1.0)
            sc_row = const.tile([1, 128], BF)
            nc.vector.memset(sc_row[:], 1.0 / WSC)
            eps128 = const.tile([128, 1], F32)
            nc.vector.memset(eps128[:], LN_EPS)
            masks_sb = [const.tile([128, 512], F8, name=f"mask{m}")
                        for m in range(4)]

            def load_masks():
                for m in range(4):
                    nc.sync.dma_start(masks_sb[m][:], masks_in.ap()[m])

            # ---- weights resident in SBUF ----
            # wqkv_sb[r, p, m, i, c] = WSC * w_eff[p*256 + i*128 + r,
            #                                      m*128 + c], fp8
            # (pair k-tiles contiguous for dual-fp8 ldweights)
            wqkv_sb = wpool.tile([128, ND // 2, 3, 2, 128], F8, name="wqkv8")
            nc.sync.dma_start(wqkv_sb[:], wqkv_in.ap())
            nws_sb = []
            for m in range(3):
                nt_ = wpool.tile([128, 1], F32, name=f"nws{m}")
                nc.sync.dma_start(nt_[:],
                                  nws_in.ap()[m * 128:(m + 1) * 128, :])
                nws_sb.append(nt_)
            bqkv_sb = []
            for m in range(3):
                bt = wpool.tile([128, 1], F32, name=f"bqkv{m}")
                nc.sync.dma_start(bt[:], bqkv_in.ap()[m * 128:(m + 1) * 128, :])
                bqkv_sb.append(bt)
            NM = 4 * D // 128       # 32 hidden chunks (full MLP per core)
            wout_sb = wpool.tile([DLOC, D], BF)
            b2b_sb = wpool.tile([128, D], BF, name="b2b")

            def load_late_weights():
                nc.sync.dma_start(wout_sb[:], wout_in.ap())
                nc.sync.dma_start(b2b_sb[:], b2b_in.ap())
                for q in range(4):
                    nc.sync.dma_start(
                        xsv[:, q, :], xsh_in.ap()[q * 128:(q + 1) * 128, :])

            resid_pool_cm = tc.tile_pool(name="resid", bufs=1)
            resid_pool = resid_pool_cm.__enter__()
            xsv = resid_pool.tile([128, 4, D], F32)   # my shard of x
            x2_sb = resid_pool.tile([128, 4, D], F32)
            h2T = resid_pool.tile([128, ND, 512], BF)     # LN2(x2)^T shard
            g1_sb = resid_pool.tile([128, NM, 512], BF)   # gelu acts [h, tok]

            # ========== stage 0: shard-local LN1 stats + tiny AG ==========
            s0x_cm = tc.tile_pool(name="s0x", bufs=1)
            s0x = s0x_cm.__enter__()
            xsb = s0x.tile([128, 4, D], BF)    # bf16 x copy for LN1 stats
            s0_cm = tc.tile_pool(name="s0", bufs=2)
            s0 = s0_cm.__enter__()
            ps0_cm = tc.tile_pool(name="ps0", bufs=2, space="PSUM")
            ps0 = ps0_cm.__enter__()
            # absorb first-collective init latency with a no-dep dummy
            wtile = s0.tile([8, 16], BF, tag="wtile", name="wtile")
            nc.vector.memset(wtile[:], 0.0)
            nc.scalar.dma_start(warm_in[:], wtile[:])
            nc.gpsimd.collective_compute(
                "AllGather", mybir.AluOpType.bypass, replica_groups=rg,
                ins=[warm_in[:].opt()], outs=[warm_out[:].opt()])
            for q in range(4):
                nc.gpsimd.dma_start(
                    xsb[:, q, :], xshb_in.ap()[q * 128:(q + 1) * 128, :])
                stats = s0.tile([128, 2, 6], F32, tag="stats", name="stats")
                xv = xsb[:, q, :].rearrange("p (s f) -> p s f", s=2)
                for s in range(2):
                    nc.vector.bn_stats(stats[:, s, :], xv[:, s, :])
                mv = s0.tile([128, 2], F32, tag="mv", name="mv")
                nc.vector.bn_aggr(mv[:], stats[:])
                rstd0 = s0.tile([128, 1], F32, tag="rstd0", name="rstd0")
                nc.scalar.activation(rstd0[:], mv[:, 1:2], AF.Sqrt,
                                     bias=eps128[:])
                nc.vector.reciprocal_approx_fast(rstd0[:], rstd0[:])
                st2 = s0.tile([128, 2], BF, tag="st2", name="st2")
                nc.vector.tensor_copy(st2[:, 0:1], mv[:, 0:1])
                nc.vector.tensor_copy(st2[:, 1:2], rstd0[:])
                stp = ps0.tile([2, 128], BF, tag="stp", name="stp")
                nc.tensor.transpose(stp[:], st2[:], ident_bf[:])
                sts = s0.tile([2, 128], BF, tag="sts", name="sts")
                nc.vector.tensor_copy(sts[:], stp[:])
                nc.scalar.dma_start(st_ag_in[2 * q:2 * q + 2, :], sts[:])
            nc.gpsimd.collective_compute(
                "AllGather", mybir.AluOpType.bypass, replica_groups=rg,
                ins=[st_ag_in[:].opt()], outs=[st_ag_out[:].opt()])
            ps0_cm.__exit__(None, None, None)
            s0_cm.__exit__(None, None, None)
            s0x_cm.__exit__(None, None, None)

            # st_ag_out rows: c*8 + q*2 + {0:mean, 1:rstd}
            st_view = st_ag_out[:].rearrange("(c x) f -> c x f", x=8)

            praws = {}

            def do_s1_mm(tt):
                q4, h4 = tt // 2, tt % 2
                xts = s1_x.tile([128, ND, 512], F8, tag="xts")
                eng = (nc.sync, nc.scalar)[tt % 2]
                eng.dma_start(
                    xts[:],
                    xt_in.ap()[:, tt * 512:(tt + 1) * 512].rearrange(
                        "(c p) t -> p c t", p=128))
                praws[tt] = (None, None, [])
                for m in range(3):
                    ps_q = ps_qkv.tile([128, 512], F32, tag="ps_q",
                                       name="ps_q")
                    for p in range(ND // 2):
                        nc.tensor.matmul(
                            ps_q[:],
                            wqkv_sb[:, p, m, :, :],
                            xts[:, 2 * p:2 * p + 2, :],
                            start=(p == 0), stop=(p == ND // 2 - 1),
                            perf_mode=DR)
                    praw = praw_pool.tile([128, 512], BF, tag="praw",
                                          name="praw")
                    nc.scalar.copy(praw[:], ps_q[:])
                    praws[tt][2].append(praw)

            def do_s1_fin(tt):
                q4, h4 = tt // 2, tt % 2
                _, _, praw3 = praws[tt]
                mean_bf = s1_stat.tile([1, 4, 128], BF, tag=f"mean_bf{tt}",
                                       name=f"mean_bf{tt}")
                rstd_bf = s1_stat.tile([1, 4, 128], BF, tag=f"rstd_bf{tt}",
                                       name=f"rstd_bf{tt}")
                nc.gpsimd.dma_start(
                    mean_bf[:], st_view[4 * h4:4 * h4 + 4, 2 * q4, :])
                nc.gpsimd.dma_start(
                    rstd_bf[:], st_view[4 * h4:4 * h4 + 4, 2 * q4 + 1, :])
                mean_v = mean_bf[:].rearrange("p a f -> p (a f)")
                rstd_v = rstd_bf[:].rearrange("p a f -> p (a f)")
                mr = s1_stat.tile([1, 512], BF, tag="mr", name="mr")
                nc.vector.tensor_mul(mr[:], mean_v, rstd_v)
                rstd_b = ps_st.tile([128, 512], F32, tag="pst",
                                    name="rstd_b")
                nc.tensor.matmul(rstd_b[:], sc_row[:], rstd_v,
                                 start=True, stop=True)
                rstd_bc = s1_tmp.tile([128, 512], BF, tag="rstd_bc")
                if tt >= 4:
                    nc.scalar.copy(rstd_bc[:], rstd_b[:])
                else:
                    nc.vector.tensor_copy(rstd_bc[:], rstd_b[:])
                mr_b = ps_st.tile([128, 512], F32, tag="pst", name="mr_b")
                nc.tensor.matmul(mr_b[:], ones_row[:], mr[:],
                                 start=True, stop=True)
                for m in range(3):
                    u = s1_tmp.tile([128, 512], BF, tag="pre", name="u")
                    nc.vector.tensor_mul(u[:], praw3[m][:], rstd_bc[:])
                    pre = s1_tmp.tile([128, 512], BF, tag="pre2",
                                      name="pre2")
                    nc.vector.scalar_tensor_tensor(
                        out=pre[:], in0=mr_b[:], scalar=nws_sb[m][:],
                        in1=u[:], op0=mybir.AluOpType.mult,
                        op1=mybir.AluOpType.add)
                    nc.vector.tensor_scalar(
                        out=qkvT[m][:, tt * 512:(tt + 1) * 512], in0=pre[:],
                        scalar1=bqkv_sb[m][:], scalar2=None,
                        op0=mybir.AluOpType.add)
                del praws[tt]
            # ============ stage 2/3/4 pools ============
            s2_vaug_cm = tc.tile_pool(name="s2_vaug", bufs=1)
            s2_vaug = s2_vaug_cm.__enter__()
            s2_exp_cm = tc.tile_pool(name="s2_exp", bufs=2)
            s2_exp = s2_exp_cm.__enter__()
            s2_misc_cm = tc.tile_pool(name="s2_misc", bufs=1)
            s2_misc = s2_misc_cm.__enter__()
            s3_r1_cm = tc.tile_pool(name="s3_r1", bufs=2)
            s3_r1 = s3_r1_cm.__enter__()
            s4_t_cm = tc.tile_pool(name="s4_t", bufs=1)
            s4_t = s4_t_cm.__enter__()
            ps_st_cm = tc.tile_pool(name="ps_st", bufs=3, space="PSUM")
            ps_st = ps_st_cm.__enter__()
            ps_o_cm = tc.tile_pool(name="ps_o", bufs=1, space="PSUM")
            ps_o = ps_o_cm.__enter__()
            ps_vt_cm = tc.tile_pool(name="ps_vt", bufs=1, space="PSUM")
            ps_vt = ps_vt_cm.__enter__()

            # persistent activations
            attn_pool_cm = tc.tile_pool(name="attn", bufs=1)
            attn_pool = attn_pool_cm.__enter__()
            qkvT = []
            for m in range(3):
                t_ = attn_pool.tile([128, T], BF, name=f"qkvT{m}")
                qkvT.append(t_)
            oT = attn_pool.tile([128, T], BF)

            # ================= stage 1 pools (popped mid-kernel) ==========
            s1_x_cm = tc.tile_pool(name="s1_x", bufs=2)
            s1_x = s1_x_cm.__enter__()
            s1_tmp_cm = tc.tile_pool(name="s1_tmp", bufs=3)
            s1_tmp = s1_tmp_cm.__enter__()
            s1_stat_cm = tc.tile_pool(name="s1_stat", bufs=1)
            s1_stat = s1_stat_cm.__enter__()
            ps_qkv_cm = tc.tile_pool(name="ps_qkv", bufs=2, space="PSUM")
            ps_qkv = ps_qkv_cm.__enter__()
            praw_pool_cm = tc.tile_pool(name="s1_praw", bufs=24)
            praw_pool = praw_pool_cm.__enter__()

            vaugs = {}

            def do_vaug(b, tl):
                """V-transposes for 512-token tile tl (4 k-chunks) of batch b."""
                tok0 = b * L
                if b not in vaugs:
                    vaug = s2_vaug.tile([128, HL, L // 256, 2, DH + 16], F8,
                                        tag=f"vaug{b}", name=f"vaug{b}")
                    nc.vector.memset(vaug[:, :, :, :, DH:DH + 1], 1.0)
                    nc.vector.memset(vaug[:, :, :, :, DH + 1:DH + 16], 0.0)
                    vaugs[b] = vaug
                vaug = vaugs[b]
                for hl in range(HL):
                    hrow = hl * DH
                    vT_u = qkvT[2][hrow:hrow + DH, tok0:tok0 + L]
                    for kc in range(4 * tl, 4 * tl + 4):
                        pv = ps_vt.tile([128, DH], BF, tag="pv",
                                        name="pv")
                        nc.tensor.transpose(
                            pv[:], vT_u[:, kc * 128:(kc + 1) * 128],
                            ident_bf[hrow:hrow + DH, hrow:hrow + DH])
                        if b == 0:
                            nc.scalar.copy(
                                vaug[:, hl, kc // 2, kc % 2, 0:DH], pv[:])
                        else:
                            nc.vector.tensor_copy(
                                vaug[:, hl, kc // 2, kc % 2, 0:DH], pv[:])

            def do_attn(b, js, fill=None):
                tok0 = b * L
                vaug = vaugs[b]
                for j in js:
                    nk = 4 * (j + 1)
                    po = [ps_o.tile([DH + 16, 512], F32, tag=f"po{hl}",
                                    name=f"po{hl}") for hl in range(HL)]
                    for kcp in range(nk // 2):
                        kc0 = 2 * kcp
                        dm0 = kc0 - (nk - 4)
                        col0 = 128 * dm0 if dm0 > 0 else 0
                        w = 512 - col0
                        ests = []
                        for hl in range(HL):
                            hrow = hl * DH
                            qsl = qkvT[0][hrow:hrow + DH,
                                          tok0 + j * 512 + col0:
                                          tok0 + (j + 1) * 512]
                            est = s2_exp.tile([128, 2, 512], F8,
                                              tag=f"est{hl}", name=f"est{hl}")
                            for i in range(2):
                                kc = kc0 + i
                                dm = kc - (nk - 4)
                                ksl = qkvT[1][hrow:hrow + DH,
                                              tok0 + kc * 128:
                                              tok0 + (kc + 1) * 128]
                                pst = ps_st.tile([128, 512], F32, tag="pst",
                                                 name="pst")
                                nc.tensor.matmul(pst[:, :w], ksl, qsl,
                                                 start=True, stop=True,
                                                 tile_position=(hrow, 0))
                                if dm >= 0:
                                    mw = 128 * (dm + 1) - col0
                                    nc.vector.tensor_add(
                                        pst[:, :mw], pst[:, :mw],
                                        masks_sb[dm][:, col0:col0 + mw])
                                nc.scalar.activation(est[:, i, :w],
                                                     pst[:, :w],
                                                     AF.Exp, scale=0.125)
                            ests.append(est)
                        for hl in range(HL):
                            nc.tensor.matmul(po[hl][:, col0:],
                                             vaug[:, hl, kcp, :, :],
                                             ests[hl][:, :, :w],
                                             start=(kcp == 0),
                                             stop=(kcp == nk // 2 - 1),
                                             perf_mode=DR)
                        if fill is not None:
                            fill()
                    for hl in range(HL):
                        hrow = hl * DH
                        den = s2_misc.tile([1, 512], F32, tag="den",
                                           name="den")
                        nc.vector.tensor_copy(den[:], po[hl][DH:DH + 1, :])
                        rec1 = s2_misc.tile([1, 512], F32, tag="rec1",
                                            name="rec1")
                        nc.vector.reciprocal_approx_fast(rec1[:], den[:])
                        rec1b = s2_misc.tile([1, 512], BF, tag="rec1b",
                                             name="rec1b")
                        nc.vector.tensor_copy(rec1b[:], rec1[:])
                        rec_b = ps_vt.tile([64, 512], F32, tag="pv",
                                           name="rec_b")
                        nc.tensor.matmul(rec_b[:], ones_row[0:1, 0:64],
                                         rec1b[:], start=True, stop=True)
                        rec_sb = s2_misc.tile([64, 512], BF, tag="rec_sb",
                                              name="rec_sb")
                        nc.vector.tensor_copy(rec_sb[:], rec_b[:])
                        nc.vector.tensor_mul(
                            oT[hrow:hrow + DH,
                               tok0 + j * 512:tok0 + (j + 1) * 512],
                            po[hl][0:DH, :], rec_sb[:])

            def do_oproj(q):
                """out-projection for quarter q (tokens q*1024..+1024) + RS."""
                for tch in range(8):
                    row0 = q * 1024 + tch * 128
                    r1 = s3_r1.tile([128, D], BF, tag="r1", name="r1")
                    for n in range(2):
                        pop = ps_st.tile([128, 512], F32, tag="pst",
                                         name="pop")
                        nc.tensor.matmul(pop[:], oT[:, row0:row0 + 128],
                                         wout_sb[:, n * 512:(n + 1) * 512],
                                         start=True, stop=True)
                        if n == 0:
                            nc.vector.tensor_copy(
                                r1[:, n * 512:(n + 1) * 512], pop[:])
                        else:
                            nc.scalar.copy(
                                r1[:, n * 512:(n + 1) * 512], pop[:])
                    nc.gpsimd.dma_start(
                        rs1_in[q][tch * 128:(tch + 1) * 128, :], r1[:])
                nc.gpsimd.collective_compute(
                    "ReduceScatter", mybir.AluOpType.add, replica_groups=rg,
                    ins=[rs1_in[q][:].opt()], outs=[rs1_out[q][:].opt()])

            def do_s4(q):
                """residual + LN2 + transpose for my piece of quarter q."""
                r1s = s4_t.tile([128, D], BF, tag="r1s", name="r1s")
                nc.sync.dma_start(r1s[:], rs1_out[q][:])
                nc.vector.tensor_add(x2_sb[:, q, :], xsv[:, q, :], r1s[:])
                stats = s4_t.tile([128, 2, 6], F32, tag="stats", name="stats")
                x2v = x2_sb[:, q, :].rearrange("p (s f) -> p s f", s=2)
                for s in range(2):
                    nc.vector.bn_stats(stats[:, s, :], x2v[:, s, :])
                mv = s4_t.tile([128, 2], F32, tag="mv", name="mv")
                nc.vector.bn_aggr(mv[:], stats[:])
                rstd2 = s4_t.tile([128, 1], F32, tag="rstd2", name="rstd2")
                nc.scalar.activation(rstd2[:], mv[:, 1:2], AF.Sqrt,
                                     bias=eps128[:])
                nc.vector.reciprocal_approx_fast(rstd2[:], rstd2[:])
                h2 = s4_t.tile([128, D], F32, tag="h2", name="h2")
                nc.vector.tensor_scalar(
                    out=h2[:], in0=x2_sb[:, q, :], scalar1=mv[:, 0:1],
                    scalar2=rstd2[:], op0=mybir.AluOpType.subtract,
                    op1=mybir.AluOpType.mult)
                for d in range(ND):
                    pt = ps_vt.tile([128, 128], F32, tag="pv", name="pt")
                    nc.tensor.transpose(
                        pt[:], h2[:, d * 128:(d + 1) * 128], ident[:])
                    nc.vector.tensor_copy(h2T[:, d, q * 128:(q + 1) * 128],
                                          pt[:])

            # ---------------- pipelined schedule (front) ----------------
            for tt in range(NT):
                do_s1_mm(tt)
            load_masks()
            load_late_weights()
            for tt in range(4):
                do_s1_fin(tt)
                do_vaug(0, tt)
            do_attn(0, (0, 1))
            do_oproj(0)
            do_attn(0, (2, 3))
            do_oproj(1)
            for tt in range(4, NT):
                do_s1_fin(tt)
                do_vaug(1, tt - 4)

            # s1 done: free its SBUF/PSUM, bring in the full w1 for DP-MLP
            praw_pool_cm.__exit__(None, None, None)
            for cm in (ps_qkv_cm, s1_stat_cm, s1_tmp_cm, s1_x_cm):
                cm.__exit__(None, None, None)
            w1p_cm = tc.tile_pool(name="w1p", bufs=1)
            w1p = w1p_cm.__enter__()
            w1_sb = [w1p.tile([128, 4 * D], BF, name=f"w1_{d}")
                     for d in range(ND)]
            b1g_sb = w1p.tile([128, NM], F32, name="b1g")
            nc.sync.dma_start(
                b1g_sb[:],
                b1g_in.ap().rearrange("(m r) o -> r (m o)", r=128))
            for d in range(ND):
                nc.sync.dma_start(w1_sb[d][:],
                                  w1_in.ap()[d * 128:(d + 1) * 128, :])
            ps_m1_cm = tc.tile_pool(name="ps_m1", bufs=2, space="PSUM")
            ps_m1 = ps_m1_cm.__enter__()

            def do_mlp1(qp, ms, raw=False):
                """MLP1 for token half qp (256 cols), hidden chunks ms.
                raw=True defers GELU: PSUM is copied to g1 by vector and a
                later batched gelu pass applies the activation in place
                (avoids exp<->gelu ACT-table thrash mid-attention)."""
                c0 = qp * 256
                for m in ms:
                    pm1 = ps_m1.tile([128, 256], F32, tag="pm1", name="pm1")
                    for d in range(ND):
                        nc.tensor.matmul(
                            pm1[:], w1_sb[d][:, m * 128:(m + 1) * 128],
                            h2T[:, d, c0:c0 + 256], start=(d == 0),
                            stop=(d == ND - 1))
                    if raw:
                        nc.vector.tensor_copy(g1_sb[:, m, c0:c0 + 256],
                                              pm1[:])
                    else:
                        nc.scalar.activation(g1_sb[:, m, c0:c0 + 256],
                                             pm1[:], AF.Gelu,
                                             bias=b1g_sb[:, m:m + 1])

            def do_gelu_pass(qp, ms):
                c0 = qp * 256
                for m in ms:
                    nc.scalar.activation(g1_sb[:, m, c0:c0 + 256],
                                         g1_sb[:, m, c0:c0 + 256],
                                         AF.Gelu, bias=b1g_sb[:, m:m + 1])

            do_attn(1, (2, 3))   # hides RS1_0 + RS1_1
            do_oproj(3)
            do_s4(0)
            do_s4(1)
            do_attn(1, (0, 1))   # hides RS1_3
            do_oproj(2)
            do_mlp1(0, range(0, 8))    # hides RS1_2
            do_s4(3)
            do_mlp1(0, range(8, 16))
            do_s4(2)
            do_mlp1(0, range(16, NM))
            do_mlp1(1, range(NM))

            for cm in (ps_m1_cm, w1p_cm, attn_pool_cm,
                       ps_vt_cm, ps_o_cm, ps_st_cm, s4_t_cm,
                       s3_r1_cm, s2_misc_cm, s2_exp_cm, s2_vaug_cm):
                cm.__exit__(None, None, None)

            # ---- stage 6: MLP2 (m-major, all 8 PSUM banks accumulate) ----
            s6_w2_cm = tc.tile_pool(name="s6_w2", bufs=8)
            s6_w2 = s6_w2_cm.__enter__()
            s6_o_cm = tc.tile_pool(name="s6_o", bufs=2)
            s6_o = s6_o_cm.__enter__()
            ps_m2_cm = tc.tile_pool(name="ps_m2", bufs=1, space="PSUM")
            ps_m2 = ps_m2_cm.__enter__()

            pm2 = [ps_m2.tile([128, 1024], F32, tag=f"pm2_{tc_}",
                              name=f"pm2_{tc_}") for tc_ in range(4)]
            for m in range(NM):
                w2c = s6_w2.tile([128, D], BF, tag="w2c", name="w2c")
                nc.sync.dma_start(w2c[:],
                                   w2_in.ap()[m * 128:(m + 1) * 128, :])
                for tc_ in range(4):
                    for n2 in range(2):
                        nc.tensor.matmul(
                            pm2[tc_][:, n2 * 512:(n2 + 1) * 512],
                            g1_sb[:, m, tc_ * 128:(tc_ + 1) * 128],
                            w2c[:, n2 * 512:(n2 + 1) * 512],
                            start=(m == 0), stop=(m == NM - 1))
            for tc_ in range(4):
                ot = s6_o.tile([128, D], F32, tag="ot", name="ot")
                nc.vector.tensor_add(ot[:], x2_sb[:, tc_, :], pm2[tc_][:])
                nc.vector.tensor_add(ot[:], ot[:], b2b_sb[:])
                nc.sync.dma_start(
                    out_ext.ap()[tc_ * 128:(tc_ + 1) * 128, :], ot[:])

            for cm in (ps_m2_cm, s6_o_cm, s6_w2_cm, resid_pool_cm):
                cm.__exit__(None, None, None)

    nc.compile()
    _CACHE["nc"] = nc
    return nc


def shard_rows(c):
    """Global token rows owned by core c (four strided pieces of 128)."""
    return np.concatenate(
        [np.arange(q * 1024 + c * 128, q * 1024 + (c + 1) * 128)
         for q in range(4)])


def make_in_maps(x, ln1_g, ln1_b, w_qkv, w_out, ln2_g, ln2_b, w1, b1, w2, b2):
    import ml_dtypes
    bf16 = ml_dtypes.bfloat16
    fp8 = ml_dtypes.float8_e4m3
    x = np.asarray(x, np.float32)
    xf = np.ascontiguousarray(x.reshape(T, D))
    xt = np.ascontiguousarray(xf.T.astype(fp8))
    w_qkv_eff = np.asarray(w_qkv) * np.asarray(ln1_g)[:, None]
    bias_qkv = np.asarray(ln1_b) @ np.asarray(w_qkv)
    w1_eff = np.asarray(w1) * np.asarray(ln2_g)[:, None]
    bias_h1 = np.asarray(ln2_b) @ np.asarray(w1) + np.asarray(b1)
    w1b = np.ascontiguousarray(w1_eff.astype(bf16))
    b1gb = np.ascontiguousarray(bias_h1, np.float32).reshape(-1, 1)
    w2b = np.ascontiguousarray(np.asarray(w2).astype(bf16))
    b2b = np.tile(np.asarray(b2).astype(bf16)[None, :], (128, 1))
    km = np.arange(128)[:, None]
    qm = np.arange(512)[None, :]
    masks = np.stack([np.where(km + 128 * m <= qm, 0.0, -448.0).astype(fp8)
                      for m in range(4)])
    in_maps = []
    for c in range(NCORES):
        cs = slice(c * DLOC, (c + 1) * DLOC)
        wq = np.concatenate(
            [w_qkv_eff[:, cs], w_qkv_eff[:, D:][:, cs],
             w_qkv_eff[:, 2 * D:][:, cs]], axis=1)
        wq8 = (wq * WSC).astype(fp8)          # [D, 384] scaled fp8
        # SBUF layout [r, p, m, i, c] = wq8[p*256 + i*128 + r, m*128 + c]
        wq8_t = np.ascontiguousarray(
            wq8.reshape(ND // 2, 2, 128, 3, 128).transpose(2, 0, 3, 1, 4))
        bq = np.concatenate(
            [bias_qkv[cs], bias_qkv[D:][cs], bias_qkv[2 * D:][cs]])
        rows = shard_rows(c)
        in_maps.append({
            "xt": xt,
            "xsh": np.ascontiguousarray(xf[rows]),
            "xshb": np.ascontiguousarray(xf[rows].astype(bf16)),
            "wqkv": wq8_t,
            "nws": np.ascontiguousarray(
                (-(wq8.astype(np.float32) / WSC).sum(axis=0)).astype(
                    np.float32)).reshape(-1, 1),
            "bqkv": np.ascontiguousarray(bq, np.float32).reshape(-1, 1),
            "wout": np.ascontiguousarray(
                np.asarray(w_out)[cs].astype(bf16)),
            "w1": w1b, "b1g": b1gb, "w2": w2b,
            "b2b": b2b,
            "masks": masks,
        })
    return in_maps


def kernel(**inputs):
    nc = build()
    in_maps = make_in_maps(**inputs)
    res = bass_utils.run_bass_kernel_spmd(
        nc, in_maps, core_ids=list(range(NCORES)))
    out = np.empty((T, D), np.float32)
    for c in range(NCORES):
        out[shard_rows(c)] = res.results[c]["out"]
    return out.reshape(B, L, D).astype(np.float32)

